# revision 9
# baseline (speedup 1.0000x reference)
"""DualGate GNN forward on 8 Trainium2 NeuronCores (Bass/Tile).

Strategy:
  - Nodes sharded across 8 cores (N/8 rows each), weights replicated.
  - Per layer, each core builds a fp16 "table" row-block [TROWS, 128] =
    (h | s=||h||^2 | pad) plus a colsum/N row, AllGathered to a replicated
    table [8*TROWS, 128] in DRAM.
  - Pass A (edges partitioned by DEST core, windows of 128 dest nodes):
    dma_gather of h_src rows + a one-hot-weighted selection matrix M
    (M[e,d] = norm_e * (dest_rel_e == d)) -> PE matmul segmented-sum into
    PSUM window tiles -> agg.  The GCN conv_w multiply is deferred to a
    dense per-node matmul (aggregation is linear).
  - Pass B (edges partitioned by SRC core): same machinery computes
    t_j = sum h_i and u_j = sum s_i over out-edges; then
    ds_j = outdeg_j*s_j - 2 h_j.t_j + u_j  (= sum_out ||h_j - h_i||^2).
  - Gates gs/gq and the node update are dense per-node-tile ops.

Self-contained: derives all sizes from input shapes; no file reads.
"""

import math
import numpy as np

NCORES = 8
P = 128
WG_A = 8    # windows per PSUM group, pass A (8*64 = 512 fp32 = 1 bank)
WG_B = 7    # pass B (7*65 = 455 <= 512)
CALL_MAX_CH = 48  # max chunks (of 128 edges) per dma_gather call
NLAYERS = 2


def _build_pass(dest, src, wgt, trow_of, NLOC, NT, NCHK, CHUNK_ROWS, WG):
    """Per-core slot arrays + a core-equalized static schedule."""
    core = dest // NLOC
    dloc = dest % NLOC
    win = dloc // P
    drel = (dloc % P).astype(np.float32)
    trow = trow_of(src)
    schk = (trow // CHUNK_ROWS).astype(np.int64)
    rel_idx = (trow % CHUNK_ROWS).astype(np.int16)

    NG = math.ceil(NT / WG)

    seg_rank_tab = np.full((NT, NCHK), -1, dtype=np.int64)
    seg_list = []
    for g in range(NG):
        for k in range(NCHK):
            for w in range(g * WG, min((g + 1) * WG, NT)):
                seg_rank_tab[w, k] = len(seg_list)
                seg_list.append((w, k))
    nseg = len(seg_list)

    seg_of_edge = seg_rank_tab[win, schk]
    cnt = np.zeros((NCORES, nseg), dtype=np.int64)
    np.add.at(cnt, (core, seg_of_edge), 1)
    eq_ch = np.ceil(cnt / P).astype(np.int64).max(axis=0)
    # every window needs >= 1 chunk so its PSUM slice gets written
    for w in range(NT):
        if sum(eq_ch[seg_rank_tab[w, k]] for k in range(NCHK)) == 0:
            eq_ch[seg_rank_tab[w, 0]] = 1

    seg_c0 = np.zeros(nseg + 1, dtype=np.int64)
    np.cumsum(eq_ch, out=seg_c0[1:])
    C = int(seg_c0[-1])
    S = C * P

    chunk_win = np.zeros(C, dtype=np.int64)
    for si, (w, k) in enumerate(seg_list):
        chunk_win[seg_c0[si]:seg_c0[si + 1]] = w
    # start/stop at GROUP granularity: start=True zeroes the whole 2KB PSUM
    # bank, so only the first matmul of a group may set it.
    chunk_start = np.zeros(C, dtype=bool)
    chunk_stop = np.zeros(C, dtype=bool)
    for g in range(NG):
        w_lo = g * WG
        w_hi = min((g + 1) * WG, NT) - 1
        gc0 = int(seg_c0[seg_rank_tab[w_lo, 0]])
        gc1 = int(seg_c0[seg_rank_tab[w_hi, NCHK - 1] + 1])
        chunk_start[gc0] = True
        chunk_stop[gc1 - 1] = True

    calls = []
    for g in range(NG):
        ws = list(range(g * WG, min((g + 1) * WG, NT)))
        for k in range(NCHK):
            c0 = int(seg_c0[seg_rank_tab[ws[0], k]])
            c1 = int(seg_c0[seg_rank_tab[ws[-1], k] + 1])
            c = c0
            while c < c1:
                nch = min(CALL_MAX_CH, c1 - c)
                calls.append((c, nch, k, g))
                c += nch

    per_core = []
    for cidx in range(NCORES):
        m = core == cidx
        er = np.nonzero(m)[0]
        order = np.argsort(seg_of_edge[er], kind="stable")
        er = er[order]
        segs = seg_of_edge[er]
        first = np.ones(len(er), dtype=bool)
        if len(er) > 1:
            first[1:] = segs[1:] != segs[:-1]
        startpos = np.nonzero(first)[0]
        runlen = np.diff(np.append(startpos, len(er)))
        within = np.arange(len(er)) - np.repeat(startpos, runlen)
        slot = seg_c0[segs] * P + within

        idx_slots = np.zeros(S, dtype=np.int16)
        dr_slots = np.zeros(S, dtype=np.float32)
        w_slots = np.zeros(S, dtype=np.float32)
        idx_slots[slot] = rel_idx[er]
        dr_slots[slot] = drel[er]
        w_slots[slot] = wgt[er]

        base = idx_slots.reshape(S // 16, 16).T      # [16, S/16]
        idxw = np.ascontiguousarray(np.tile(base, (8, 1)))
        drw = np.ascontiguousarray(dr_slots.reshape(C, P).T)
        ww = np.ascontiguousarray(w_slots.reshape(C, P).T)
        per_core.append(dict(idxw=idxw, drw=drw, ww=ww))

    sched = dict(C=C, calls=calls, chunk_win=chunk_win,
                 chunk_start=chunk_start, chunk_stop=chunk_stop)
    return sched, per_core


def kernel(x, edge_index, enc_w, enc_b, skip_w, conv_w, conv_b, dec_w, dec_b):
    import jax  # noqa: F401  (axon boot side effect)
    import concourse.bass as bass  # noqa: F401
    import concourse.mybir as mybir
    import concourse.tile as tile
    from concourse import bacc
    from concourse.bass_utils import run_bass_kernel_spmd

    FP16, FP32, I16 = mybir.dt.float16, mybir.dt.float32, mybir.dt.int16
    Alu = mybir.AluOpType
    Act = mybir.ActivationFunctionType

    x = np.asarray(x, dtype=np.float32)
    edge_index = np.asarray(edge_index)
    enc_w = np.asarray(enc_w, dtype=np.float32)
    enc_b = np.asarray(enc_b, dtype=np.float32)
    skip_w = np.asarray(skip_w, dtype=np.float32)
    conv_w = np.asarray(conv_w, dtype=np.float32)
    conv_b = np.asarray(conv_b, dtype=np.float32)
    dec_w = np.asarray(dec_w, dtype=np.float32)
    dec_b = np.asarray(dec_b, dtype=np.float32)

    N, F = x.shape
    H = enc_w.shape[1]
    OUTF = dec_w.shape[1]
    assert N % NCORES == 0 and F == 128 and H == 64
    NLOC = N // NCORES
    NT = math.ceil(NLOC / P)
    NLOC_pad = NT * P
    TROWS = NLOC_pad + P
    NCHK = 4
    CHUNK_ROWS = TROWS * NCORES // NCHK
    assert CHUNK_ROWS <= 32767 and TROWS * NCORES % NCHK == 0
    NG_A = math.ceil(NT / WG_A)  # noqa: F841
    TOT_ROWS = TROWS * NCORES

    row = np.asarray(edge_index[0], dtype=np.int64)
    col = np.asarray(edge_index[1], dtype=np.int64)

    deg = np.bincount(col, minlength=N).astype(np.float64) + 1.0
    dinv = 1.0 / np.sqrt(deg)
    norm = (dinv[row] * dinv[col]).astype(np.float32)
    outdeg_np = np.bincount(row, minlength=N).astype(np.float64)
    rdeg_np = (1.0 / (outdeg_np + 1e-10)).astype(np.float32)
    dinv2_np = (dinv * dinv).astype(np.float32)

    def trow_of(n):
        return (n // NLOC) * TROWS + (n % NLOC)

    schedA, pcA = _build_pass(col, row, norm, trow_of, NLOC, NT, NCHK,
                              CHUNK_ROWS, WG_A)
    schedB, pcB = _build_pass(row, col, np.ones(len(row), np.float32), trow_of,
                              NLOC, NT, NCHK, CHUNK_ROWS, WG_B)

    def pack_nodal(v):
        out = np.zeros((NCORES, P, NT), dtype=np.float32)
        for c in range(NCORES):
            padded = np.zeros(NLOC_pad, dtype=np.float32)
            padded[:NLOC] = v[c * NLOC:(c + 1) * NLOC]
            out[c] = padded.reshape(NT, P).T
        return out

    dinv2_pk = pack_nodal(dinv2_np)
    outdeg_pk = pack_nodal(outdeg_np.astype(np.float32))
    rdeg_pk = pack_nodal(rdeg_np)

    x_pk = np.zeros((NCORES, NLOC_pad, F), dtype=np.float32)
    for c in range(NCORES):
        x_pk[c, :NLOC] = x[c * NLOC:(c + 1) * NLOC]

    R_np = np.tile(np.arange(P, dtype=np.float16)[None, :], (P, 1))
    id128_np = np.eye(P, dtype=np.float32)
    ones128_np = np.ones((P, 1), dtype=np.float32)
    ones8_np = np.ones((NCORES, 1), dtype=np.float16)
    ones1x128_np = np.ones((1, P), dtype=np.float16)

    # ---------------- device program ----------------
    nc = bacc.Bacc("TRN2", target_bir_lowering=False, debug=False,
                   num_devices=NCORES)

    def din(name, shape, dt):
        return nc.dram_tensor(name, list(shape), dt, kind="ExternalInput").ap()

    x_e = din("x", (NLOC_pad, F), FP32)
    idxA_e = din("idxA", pcA[0]["idxw"].shape, I16)
    drA_e = din("drA", pcA[0]["drw"].shape, FP32)
    wwA_e = din("wwA", pcA[0]["ww"].shape, FP32)
    idxB_e = din("idxB", pcB[0]["idxw"].shape, I16)
    drB_e = din("drB", pcB[0]["drw"].shape, FP32)
    wwB_e = din("wwB", pcB[0]["ww"].shape, FP32)
    dinv2_e = din("dinv2", (P, NT), FP32)
    outdeg_e = din("outdeg", (P, NT), FP32)
    rdeg_e = din("rdeg", (P, NT), FP32)
    R_e = din("Riota", (P, P), FP16)
    id_e = din("id128", (P, P), FP32)
    ones128_e = din("ones128", (P, 1), FP32)
    ones8_e = din("ones8", (NCORES, 1), FP16)
    ones1x128_e = din("ones1x128", (1, P), FP16)
    encw_e = din("enc_w", (F, H), FP32)
    encb_e = din("enc_b", (H, 1), FP32)
    skipw_e = din("skip_w", (F, H), FP32)
    convw_e = din("conv_w", (H, H), FP32)
    convb_e = din("conv_b", (H, 1), FP32)
    decw_e = din("dec_w", (H, OUTF), FP32)
    decb_e = din("dec_b", (OUTF, 1), FP32)
    out_e = nc.dram_tensor("out", [NLOC_pad, OUTF], FP32,
                           kind="ExternalOutput").ap()
    import os as _os
    KDBG = bool(_os.environ.get("KDBG"))
    if KDBG:
        dbg_h0 = nc.dram_tensor("dbg_h0", [P, NT * H], FP32, kind="ExternalOutput").ap()
        dbg_agg = nc.dram_tensor("dbg_agg", [P, NT * H], FP32, kind="ExternalOutput").ap()
        dbg_tu = nc.dram_tensor("dbg_tu", [P, NT * (H + 1)], FP32, kind="ExternalOutput").ap()
        dbg_gmb = nc.dram_tensor("dbg_gmb", [P, H], FP32, kind="ExternalOutput").ap()
        dbg_h1 = nc.dram_tensor("dbg_h1", [P, NT * H], FP32, kind="ExternalOutput").ap()

    aginp = [nc.dram_tensor(f"aginp{l}", [TROWS, P], FP16)
             for l in range(NLAYERS)]
    tableAG = [nc.dram_tensor(f"tableAG{l}", [TOT_ROWS, P], FP16,
                              addr_space="Shared") for l in range(NLAYERS)]

    with tile.TileContext(nc) as tc:
        with (
            tc.tile_pool(name="const", bufs=1) as cpool,
            tc.tile_pool(name="slab", bufs=1) as slab,
            tc.tile_pool(name="io", bufs=3) as io,
            tc.tile_pool(name="mpool", bufs=4) as mpool,
            tc.tile_pool(name="gpool", bufs=2) as gpool,
            tc.tile_pool(name="meta", bufs=2) as meta,
            tc.tile_pool(name="scr", bufs=3) as scr,
            tc.tile_pool(name="ps_big", bufs=2, space="PSUM") as ps_big,
            tc.tile_pool(name="ps_t", bufs=2, space="PSUM") as ps_t,
            tc.tile_pool(name="ps_f", bufs=2, space="PSUM") as ps_f,
            tc.tile_pool(name="ps_s", bufs=1, space="PSUM") as ps_s,
        ):
            def load_const(ext, shape, dt):
                t = cpool.tile(list(shape), dt, name=ext.tensor.name + "_sb")
                nc.sync.dma_start(out=t[:], in_=ext[:, :])
                return t

            R_t = load_const(R_e, (P, P), FP16)
            id_t = load_const(id_e, (P, P), FP32)
            ones128_t = load_const(ones128_e, (P, 1), FP32)
            ones8_t = load_const(ones8_e, (NCORES, 1), FP16)
            ones1x128_t = load_const(ones1x128_e, (1, P), FP16)
            encw_t = load_const(encw_e, (F, H), FP32)
            encb_t = load_const(encb_e, (H, 1), FP32)
            skipw_t = load_const(skipw_e, (F, H), FP32)
            convw_t = load_const(convw_e, (H, H), FP32)
            convb_t = load_const(convb_e, (H, 1), FP32)
            decw_t = load_const(decw_e, (H, OUTF), FP32)
            decb_t = load_const(decb_e, (OUTF, 1), FP32)
            dinv2_t = load_const(dinv2_e, (P, NT), FP32)
            outdeg_t = load_const(outdeg_e, (P, NT), FP32)
            rdeg_t = load_const(rdeg_e, (P, NT), FP32)

            h_sl = slab.tile([P, NT * H], FP32)
            xskip_sl = slab.tile([P, NT * H], FP32)
            agg_sl = slab.tile([P, NT * H], FP32)
            tu_sl = slab.tile([P, NT * (H + 1)], FP32)
            s_sl = slab.tile([P, NT], FP32)
            gmb_sl = slab.tile([P, H], FP32)
            zero16 = cpool.tile([P, P], FP16)
            nc.vector.memset(zero16[:], 0)

            def hsl(t):
                return h_sl[:, t * H:(t + 1) * H]

            def xssl(t):
                return xskip_sl[:, t * H:(t + 1) * H]

            def aggsl(t):
                return agg_sl[:, t * H:(t + 1) * H]

            def tsl(t):
                return tu_sl[:, t * (H + 1):t * (H + 1) + H]

            def usl(t):
                return tu_sl[:, t * (H + 1) + H:(t + 1) * (H + 1)]

            def pt_tile():
                return ps_t.tile([P, P], FP32, space="PSUM", tag="pt", name="pt")

            def pf_tile():
                return ps_f.tile([P, P], FP32, space="PSUM", tag="pf", name="pf")

            # ---------- encoder ----------
            for t in range(NT):
                xt = io.tile([P, F], FP32, tag="xt")
                nc.sync.dma_start(out=xt[:], in_=x_e[t * P:(t + 1) * P, :])
                pxT = pt_tile()
                nc.tensor.transpose(pxT[:], xt[:], id_t[:])
                xT = scr.tile([P, P], FP32, tag="xT")
                nc.vector.tensor_copy(xT[:], pxT[:])

                pf = pf_tile()
                nc.tensor.matmul(pf[0:H, :], lhsT=encw_t[:], rhs=xT[:],
                                 start=True, stop=True)
                hT = scr.tile([H, P], FP32, tag="hT")
                nc.scalar.activation(hT[:], pf[0:H, :], Act.Relu,
                                     bias=encb_t[:, 0:1], scale=1.0)
                pb = pt_tile()
                nc.tensor.transpose(pb[:, 0:H], hT[:], id_t[0:H, 0:H])
                nc.vector.tensor_copy(hsl(t), pb[:, 0:H])

                pf2 = pf_tile()
                nc.tensor.matmul(pf2[0:H, :], lhsT=skipw_t[:], rhs=xT[:],
                                 start=True, stop=True)
                sT = scr.tile([H, P], FP32, tag="hT")
                nc.vector.tensor_copy(sT[:], pf2[0:H, :])
                pb2 = pt_tile()
                nc.tensor.transpose(pb2[:, 0:H], sT[:], id_t[0:H, 0:H])
                nc.vector.tensor_copy(xssl(t), pb2[:, 0:H])

            if KDBG:
                nc.sync.dma_start(out=dbg_h0[:, :], in_=h_sl[:])

            # ---------- layers ----------
            for l in range(NLAYERS):
                # table build
                pcs = ps_s.tile([1, H], FP32, space="PSUM", tag="pcs")
                for t in range(NT):
                    sq = scr.tile([P, H], FP32, tag="sq")
                    nc.vector.tensor_mul(sq[:], hsl(t), hsl(t))
                    nc.vector.tensor_reduce(s_sl[:, t:t + 1], sq[:],
                                            axis=mybir.AxisListType.X,
                                            op=Alu.add)
                    tb = io.tile([P, P], FP16, tag="tb")
                    nc.vector.tensor_copy(tb[:, 0:H], hsl(t))
                    nc.vector.tensor_copy(tb[:, H:H + 1], s_sl[:, t:t + 1])
                    nc.vector.tensor_copy(tb[:, H + 1:P], zero16[:, H + 1:P])
                    nc.sync.dma_start(out=aginp[l][t * P:(t + 1) * P, :],
                                      in_=tb[:])
                    nc.tensor.matmul(pcs[:], lhsT=ones128_t[:], rhs=hsl(t),
                                     start=(t == 0), stop=(t == NT - 1),
                                     skip_group_check=True)
                csum = scr.tile([1, H], FP32, tag="csum")
                nc.scalar.activation(csum[:], pcs[:], Act.Copy, bias=0.0,
                                     scale=1.0 / N)
                csrow = io.tile([1, P], FP16, tag="csrow")
                nc.vector.tensor_copy(csrow[:, 0:H], csum[:])
                nc.vector.tensor_copy(csrow[:, H:P], zero16[0:1, H:P])
                nc.sync.dma_start(out=aginp[l][NLOC_pad:NLOC_pad + 1, :],
                                  in_=csrow[:])
                nc.sync.dma_start(out=aginp[l][NLOC_pad + 1:NLOC_pad + P, :],
                                  in_=zero16[0:P - 1, :])

                nc.gpsimd.collective_compute(
                    "AllGather", Alu.bypass,
                    replica_groups=[list(range(NCORES))],
                    ins=[aginp[l][:, :]],
                    outs=[tableAG[l][:, :]],
                )

                # gmean from the 8 colsum/N rows
                gm8 = scr.tile([NCORES, H], FP16, tag="gm8")
                tview = tableAG[l][:, :].rearrange("(r t) e -> r t e", r=NCORES)
                nc.sync.dma_start(out=gm8[:], in_=tview[:, NLOC_pad, 0:H])
                pgm = ps_s.tile([1, H], FP32, space="PSUM", tag="pcs")
                nc.tensor.matmul(pgm[:], lhsT=ones8_t[:], rhs=gm8[:],
                                 start=True, stop=True)
                gmrow = scr.tile([1, H], FP16, tag="gmrow")
                nc.vector.tensor_copy(gmrow[:], pgm[:])
                pgb = pf_tile()
                nc.tensor.matmul(pgb[:, 0:H], lhsT=ones1x128_t[:], rhs=gmrow[:],
                                 start=True, stop=True)
                nc.vector.tensor_copy(gmb_sl[:], pgb[:, 0:H])

                # ---- gather/segsum passes ----
                def run_pass(sched, idx_e, dr_e, ww_e, WG, FW, out_slab_fn):
                    chunk_win = sched["chunk_win"]
                    cst, csp = sched["chunk_start"], sched["chunk_stop"]
                    state = dict(g=-1, pgrp=None, gw=0)

                    def evac():
                        g, gw, pg = state["g"], state["gw"], state["pgrp"]
                        w0 = g * WG
                        nc.scalar.activation(out_slab_fn(w0, gw),
                                             pg[:, 0:gw * FW], Act.Copy,
                                             bias=0.0, scale=1.0)

                    for (c0, nch, k, g) in sched["calls"]:
                        if g != state["g"]:
                            if state["pgrp"] is not None:
                                evac()
                            state["g"] = g
                            state["gw"] = min(WG, NT - g * WG)
                            state["pgrp"] = ps_big.tile([P, 512], FP32, space="PSUM", tag="pgrp", name="pgrp")
                        pgrp = state["pgrp"]
                        L = nch * P
                        idx_t = meta.tile([P, CALL_MAX_CH * 8], I16, tag="idx")
                        nc.sync.dma_start(out=idx_t[:, 0:nch * 8],
                                          in_=idx_e[:, c0 * 8:(c0 + nch) * 8])
                        dr_t = meta.tile([P, CALL_MAX_CH], FP32, tag="dr")
                        nc.sync.dma_start(out=dr_t[:, 0:nch],
                                          in_=dr_e[:, c0:c0 + nch])
                        w_t = meta.tile([P, CALL_MAX_CH], FP32, tag="wt")
                        nc.sync.dma_start(out=w_t[:, 0:nch],
                                          in_=ww_e[:, c0:c0 + nch])
                        g_t = gpool.tile([P, CALL_MAX_CH * P], FP16, tag="gt")
                        g3 = g_t[:].rearrange("p (c e) -> p c e", c=CALL_MAX_CH)
                        nc.gpsimd.dma_gather(
                            out_ap=g3[:, 0:nch, :],
                            in_ap=tableAG[l][k * CHUNK_ROWS:(k + 1) * CHUNK_ROWS, :],
                            idxs_ap=idx_t[:, 0:nch * 8],
                            num_idxs=L, num_idxs_reg=L, elem_size=P,
                            single_packet=False,
                        )
                        for j in range(nch):
                            cj = c0 + j
                            w = int(chunk_win[cj])
                            wloc = w - g * WG
                            m_t = mpool.tile([P, P], FP16, tag="m")
                            nc.vector.tensor_scalar(
                                out=m_t[:], in0=R_t[:],
                                scalar1=dr_t[:, j:j + 1],
                                scalar2=w_t[:, j:j + 1],
                                op0=Alu.is_equal, op1=Alu.mult)
                            nc.tensor.matmul(
                                pgrp[:, wloc * FW:wloc * FW + FW],
                                lhsT=m_t[:], rhs=g3[:, j, 0:FW],
                                start=bool(cst[cj]), stop=bool(csp[cj]),
                                skip_group_check=True)
                    if state["pgrp"] is not None:
                        evac()

                def aggout(w0, gw):
                    return agg_sl[:, w0 * H:(w0 + gw) * H]

                def tuout(w0, gw):
                    return tu_sl[:, w0 * (H + 1):(w0 + gw) * (H + 1)]

                run_pass(schedA, idxA_e, drA_e, wwA_e, WG_A, H, aggout)
                run_pass(schedB, idxB_e, drB_e, wwB_e, WG_B, H + 1, tuout)
                if KDBG and l == 0:
                    nc.sync.dma_start(out=dbg_agg[:, :], in_=agg_sl[:])
                    nc.sync.dma_start(out=dbg_tu[:, :], in_=tu_sl[:])
                    nc.sync.dma_start(out=dbg_gmb[:, :], in_=gmb_sl[:])

                # ---- dense per-tile update ----
                for t in range(NT):
                    q = scr.tile([P, H], FP32, tag="q")
                    nc.vector.tensor_scalar(out=q[:], in0=hsl(t),
                                            scalar1=dinv2_t[:, t:t + 1],
                                            scalar2=None, op0=Alu.mult)
                    nc.vector.tensor_add(q[:], q[:], aggsl(t))
                    pq = pt_tile()
                    nc.tensor.transpose(pq[0:H, :], q[:], id_t[:])
                    qT = scr.tile([H, P], FP32, tag="qT")
                    nc.vector.tensor_copy(qT[:], pq[0:H, :])
                    pxa = pf_tile()
                    nc.tensor.matmul(pxa[0:H, :], lhsT=convw_t[:], rhs=qT[:],
                                     start=True, stop=True)
                    xaT = scr.tile([H, P], FP32, tag="xaT")
                    nc.scalar.activation(xaT[:], pxa[0:H, :], Act.Relu,
                                         bias=convb_t[:, 0:1], scale=1.0)
                    pxa2 = pt_tile()
                    nc.tensor.transpose(pxa2[:, 0:H], xaT[:], id_t[0:H, 0:H])
                    xagg = scr.tile([P, H], FP32, tag="xagg")
                    nc.vector.tensor_copy(xagg[:], pxa2[:, 0:H])

                    ht = scr.tile([P, H], FP32, tag="ht")
                    nc.vector.tensor_mul(ht[:], hsl(t), tsl(t))
                    hdot = scr.tile([P, 1], FP32, tag="hdot")
                    nc.vector.tensor_reduce(hdot[:], ht[:],
                                            axis=mybir.AxisListType.X,
                                            op=Alu.add)
                    ds = scr.tile([P, 1], FP32, tag="ds")
                    nc.vector.tensor_scalar(out=ds[:], in0=hdot[:],
                                            scalar1=-2.0, scalar2=None,
                                            op0=Alu.mult)
                    nc.vector.tensor_add(ds[:], ds[:], usl(t))
                    os_ = scr.tile([P, 1], FP32, tag="os")
                    nc.vector.tensor_scalar(out=os_[:], in0=s_sl[:, t:t + 1],
                                            scalar1=outdeg_t[:, t:t + 1],
                                            scalar2=None, op0=Alu.mult)
                    nc.vector.tensor_add(ds[:], ds[:], os_[:])
                    gs = scr.tile([P, 1], FP32, tag="gs")
                    nc.scalar.activation(gs[:], ds[:], Act.Tanh,
                                         scale=rdeg_t[:, t:t + 1])

                    df = scr.tile([P, H], FP32, tag="df")
                    nc.vector.tensor_sub(df[:], hsl(t), gmb_sl[:])
                    av = scr.tile([P, H], FP32, tag="av")
                    nc.scalar.activation(av[:], df[:], Act.Abs)
                    cv = scr.tile([P, H], FP32, tag="cv")
                    nc.scalar.activation(cv[:], av[:], Act.Sqrt)
                    nc.vector.tensor_mul(cv[:], cv[:], av[:])   # a^1.5
                    nc.vector.tensor_mul(cv[:], cv[:], av[:])   # a^2.5
                    dq = scr.tile([P, 1], FP32, tag="dq")
                    nc.vector.tensor_reduce(dq[:], cv[:],
                                            axis=mybir.AxisListType.X,
                                            op=Alu.add)
                    gq = scr.tile([P, 1], FP32, tag="gq")
                    nc.scalar.activation(gq[:], dq[:], Act.Tanh)
                    nc.vector.tensor_scalar(out=gq[:], in0=gq[:], scalar1=-1.0,
                                            scalar2=1.0, op0=Alu.mult,
                                            op1=Alu.add)

                    den = scr.tile([P, 1], FP32, tag="den")
                    nc.vector.tensor_add(den[:], gs[:], gq[:])
                    nc.vector.tensor_scalar(out=den[:], in0=den[:],
                                            scalar1=1.0, scalar2=None,
                                            op0=Alu.add)
                    rec = scr.tile([P, 1], FP32, tag="rec")
                    nc.vector.reciprocal(rec[:], den[:])

                    u1 = scr.tile([P, H], FP32, tag="u1")
                    nc.vector.tensor_scalar(out=u1[:], in0=xagg[:],
                                            scalar1=gs[:, 0:1], scalar2=None,
                                            op0=Alu.mult)
                    u2 = scr.tile([P, H], FP32, tag="u2")
                    nc.vector.tensor_scalar(out=u2[:], in0=xssl(t),
                                            scalar1=gq[:, 0:1], scalar2=None,
                                            op0=Alu.mult)
                    nc.vector.tensor_add(u1[:], u1[:], u2[:])
                    nc.vector.tensor_add(u1[:], u1[:], hsl(t))
                    nc.vector.tensor_scalar(out=hsl(t), in0=u1[:],
                                            scalar1=rec[:, 0:1], scalar2=None,
                                            op0=Alu.mult)

                if KDBG and l == 0:
                    nc.sync.dma_start(out=dbg_h1[:, :], in_=h_sl[:])

            # ---------- decoder ----------
            for t in range(NT):
                ph = pt_tile()
                nc.tensor.transpose(ph[0:H, :], hsl(t), id_t[:])
                hT2 = scr.tile([H, P], FP32, tag="qT")
                nc.vector.tensor_copy(hT2[:], ph[0:H, :])
                po = pf_tile()
                nc.tensor.matmul(po[0:OUTF, :], lhsT=decw_t[:], rhs=hT2[:],
                                 start=True, stop=True)
                oT = scr.tile([OUTF, P], FP32, tag="oT")
                nc.vector.tensor_scalar(out=oT[:], in0=po[0:OUTF, :],
                                        scalar1=decb_t[:, 0:1], scalar2=None,
                                        op0=Alu.add)
                pob = pt_tile()
                nc.tensor.transpose(pob[:, 0:OUTF], oT[:], id_t[0:OUTF, 0:OUTF])
                ot = io.tile([P, OUTF], FP32, tag="ot")
                nc.vector.tensor_copy(ot[:], pob[:, 0:OUTF])
                nc.sync.dma_start(out=out_e[t * P:(t + 1) * P, :], in_=ot[:])

    nc.compile()

    in_maps = []
    for c in range(NCORES):
        in_maps.append({
            "x": x_pk[c],
            "idxA": pcA[c]["idxw"], "drA": pcA[c]["drw"], "wwA": pcA[c]["ww"],
            "idxB": pcB[c]["idxw"], "drB": pcB[c]["drw"], "wwB": pcB[c]["ww"],
            "dinv2": dinv2_pk[c], "outdeg": outdeg_pk[c], "rdeg": rdeg_pk[c],
            "Riota": R_np, "id128": id128_np, "ones128": ones128_np,
            "ones8": ones8_np, "ones1x128": ones1x128_np,
            "enc_w": enc_w, "enc_b": enc_b.reshape(H, 1),
            "skip_w": skip_w, "conv_w": conv_w,
            "conv_b": conv_b.reshape(H, 1),
            "dec_w": dec_w, "dec_b": dec_b.reshape(OUTF, 1),
        })

    import os
    if os.environ.get("KSIM"):
        from concourse import bass_interp
        sim = bass_interp.MultiCoreSim(nc, num_cores=NCORES)
        for c in range(NCORES):
            for k, v in in_maps[c].items():
                sim.cores[c].tensor(k)[:] = v
        sim.simulate(check_with_hw=False)
        if KDBG:
            kernel.dbg = [{k: np.array(sim.cores[c].tensor(k)) for k in
                           ("dbg_h0", "dbg_agg", "dbg_tu", "dbg_gmb", "dbg_h1")}
                          for c in range(NCORES)]
        out = np.concatenate([np.array(sim.cores[c].tensor("out"))[:NLOC]
                              for c in range(NCORES)], axis=0)
        return out.astype(np.float32)

    import time as _time
    t0 = _time.perf_counter()
    res = run_bass_kernel_spmd(nc, in_maps, list(range(NCORES)))
    t1 = _time.perf_counter()
    globals()["LAST_RUN_S"] = t1 - t0
    if os.environ.get("KTIME"):
        t2 = _time.perf_counter()
        res = run_bass_kernel_spmd(nc, in_maps, list(range(NCORES)))
        t3 = _time.perf_counter()
        globals()["LAST_RUN2_S"] = t3 - t2
        globals()["LAST_EXEC_NS"] = int((t3 - t2) * 1e9)
    out = np.concatenate([res.results[c]["out"][:NLOC]
                          for c in range(NCORES)], axis=0)
    return out.astype(np.float32)


# revision 10
# speedup vs baseline: 49.0968x; 49.0968x over previous
"""DualGate GNN forward on 8 Trainium2 NeuronCores (Bass/Tile).

Strategy:
  - Nodes sharded across 8 cores (N/8 rows each), weights replicated.
  - Per layer, each core builds a fp16 "table" row-block [TROWS, 128] =
    (h | s=||h||^2 | pad) plus a colsum/N row, AllGathered to a replicated
    table [8*TROWS, 128] in DRAM.
  - Pass A (edges partitioned by DEST core, windows of 128 dest nodes):
    dma_gather of h_src rows + a one-hot-weighted selection matrix M
    (M[e,d] = norm_e * (dest_rel_e == d)) -> PE matmul segmented-sum into
    PSUM window tiles -> agg.  The GCN conv_w multiply is deferred to a
    dense per-node matmul (aggregation is linear).
  - Pass B (edges partitioned by SRC core): same machinery computes
    t_j = sum h_i and u_j = sum s_i over out-edges; then
    ds_j = outdeg_j*s_j - 2 h_j.t_j + u_j  (= sum_out ||h_j - h_i||^2).
  - Gates gs/gq and the node update are dense per-node-tile ops.

Self-contained: derives all sizes from input shapes; no file reads.
"""

import math
import numpy as np

NCORES = 8
P = 128
WG_A = 8    # windows per PSUM group, pass A (8*64 = 512 fp32 = 1 bank)
WG_B = 7    # pass B (7*65 = 455 <= 512)
CALL_MAX_CH = 48  # max chunks (of 128 edges) per dma_gather call
NLAYERS = 2


def _build_pass(dest, src, wgt, trow_of, NLOC, NT, NCHK, CHUNK_ROWS, WG):
    """Per-core slot arrays + a core-equalized static schedule."""
    core = dest // NLOC
    dloc = dest % NLOC
    win = dloc // P
    drel = (dloc % P).astype(np.float32)
    trow = trow_of(src)
    schk = (trow // CHUNK_ROWS).astype(np.int64)
    rel_idx = (trow % CHUNK_ROWS).astype(np.int16)

    NG = math.ceil(NT / WG)

    seg_rank_tab = np.full((NT, NCHK), -1, dtype=np.int64)
    seg_list = []
    for g in range(NG):
        for k in range(NCHK):
            for w in range(g * WG, min((g + 1) * WG, NT)):
                seg_rank_tab[w, k] = len(seg_list)
                seg_list.append((w, k))
    nseg = len(seg_list)

    seg_of_edge = seg_rank_tab[win, schk]
    cnt = np.zeros((NCORES, nseg), dtype=np.int64)
    np.add.at(cnt, (core, seg_of_edge), 1)
    eq_ch = np.ceil(cnt / P).astype(np.int64).max(axis=0)
    # every window needs >= 1 chunk so its PSUM slice gets written
    for w in range(NT):
        if sum(eq_ch[seg_rank_tab[w, k]] for k in range(NCHK)) == 0:
            eq_ch[seg_rank_tab[w, 0]] = 1

    seg_c0 = np.zeros(nseg + 1, dtype=np.int64)
    np.cumsum(eq_ch, out=seg_c0[1:])
    C = int(seg_c0[-1])
    S = C * P

    chunk_win = np.zeros(C, dtype=np.int64)
    for si, (w, k) in enumerate(seg_list):
        chunk_win[seg_c0[si]:seg_c0[si + 1]] = w
    # start/stop at GROUP granularity: start=True zeroes the whole 2KB PSUM
    # bank, so only the first matmul of a group may set it.
    chunk_start = np.zeros(C, dtype=bool)
    chunk_stop = np.zeros(C, dtype=bool)
    for g in range(NG):
        w_lo = g * WG
        w_hi = min((g + 1) * WG, NT) - 1
        gc0 = int(seg_c0[seg_rank_tab[w_lo, 0]])
        gc1 = int(seg_c0[seg_rank_tab[w_hi, NCHK - 1] + 1])
        chunk_start[gc0] = True
        chunk_stop[gc1 - 1] = True

    calls = []
    for g in range(NG):
        ws = list(range(g * WG, min((g + 1) * WG, NT)))
        for k in range(NCHK):
            c0 = int(seg_c0[seg_rank_tab[ws[0], k]])
            c1 = int(seg_c0[seg_rank_tab[ws[-1], k] + 1])
            c = c0
            while c < c1:
                nch = min(CALL_MAX_CH, c1 - c)
                calls.append((c, nch, k, g))
                c += nch

    per_core = []
    for cidx in range(NCORES):
        m = core == cidx
        er = np.nonzero(m)[0]
        order = np.argsort(seg_of_edge[er], kind="stable")
        er = er[order]
        segs = seg_of_edge[er]
        first = np.ones(len(er), dtype=bool)
        if len(er) > 1:
            first[1:] = segs[1:] != segs[:-1]
        startpos = np.nonzero(first)[0]
        runlen = np.diff(np.append(startpos, len(er)))
        within = np.arange(len(er)) - np.repeat(startpos, runlen)
        slot = seg_c0[segs] * P + within

        idx_slots = np.zeros(S, dtype=np.int16)
        dr_slots = np.zeros(S, dtype=np.float32)
        w_slots = np.zeros(S, dtype=np.float32)
        idx_slots[slot] = rel_idx[er]
        dr_slots[slot] = drel[er]
        w_slots[slot] = wgt[er]

        base = idx_slots.reshape(S // 16, 16).T      # [16, S/16]
        idxw = np.ascontiguousarray(np.tile(base, (8, 1)))
        drw = np.ascontiguousarray(dr_slots.reshape(C, P).T)
        ww = np.ascontiguousarray(w_slots.reshape(C, P).T)
        per_core.append(dict(idxw=idxw, drw=drw, ww=ww))

    sched = dict(C=C, calls=calls, chunk_win=chunk_win,
                 chunk_start=chunk_start, chunk_stop=chunk_stop)
    return sched, per_core


def kernel(x, edge_index, enc_w, enc_b, skip_w, conv_w, conv_b, dec_w, dec_b):
    import jax  # noqa: F401  (axon boot side effect)
    import concourse.bass as bass  # noqa: F401
    import concourse.mybir as mybir
    import concourse.tile as tile
    from concourse import bacc
    from concourse.bass_utils import run_bass_kernel_spmd

    FP16, FP32, I16 = mybir.dt.float16, mybir.dt.float32, mybir.dt.int16
    Alu = mybir.AluOpType
    Act = mybir.ActivationFunctionType

    x = np.asarray(x, dtype=np.float32)
    edge_index = np.asarray(edge_index)
    enc_w = np.asarray(enc_w, dtype=np.float32)
    enc_b = np.asarray(enc_b, dtype=np.float32)
    skip_w = np.asarray(skip_w, dtype=np.float32)
    conv_w = np.asarray(conv_w, dtype=np.float32)
    conv_b = np.asarray(conv_b, dtype=np.float32)
    dec_w = np.asarray(dec_w, dtype=np.float32)
    dec_b = np.asarray(dec_b, dtype=np.float32)

    N, F = x.shape
    H = enc_w.shape[1]
    OUTF = dec_w.shape[1]
    assert N % NCORES == 0 and F == 128 and H == 64
    NLOC = N // NCORES
    NT = math.ceil(NLOC / P)
    NLOC_pad = NT * P
    TROWS = NLOC_pad + P
    NCHK = 4
    CHUNK_ROWS = TROWS * NCORES // NCHK
    assert CHUNK_ROWS <= 32767 and TROWS * NCORES % NCHK == 0
    NG_A = math.ceil(NT / WG_A)  # noqa: F841
    TOT_ROWS = TROWS * NCORES

    row = np.asarray(edge_index[0], dtype=np.int64)
    col = np.asarray(edge_index[1], dtype=np.int64)

    deg = np.bincount(col, minlength=N).astype(np.float64) + 1.0
    dinv = 1.0 / np.sqrt(deg)
    norm = (dinv[row] * dinv[col]).astype(np.float32)
    outdeg_np = np.bincount(row, minlength=N).astype(np.float64)
    rdeg_np = (1.0 / (outdeg_np + 1e-10)).astype(np.float32)
    dinv2_np = (dinv * dinv).astype(np.float32)

    def trow_of(n):
        return (n // NLOC) * TROWS + (n % NLOC)

    schedA, pcA = _build_pass(col, row, norm, trow_of, NLOC, NT, NCHK,
                              CHUNK_ROWS, WG_A)
    schedB, pcB = _build_pass(row, col, np.ones(len(row), np.float32), trow_of,
                              NLOC, NT, NCHK, CHUNK_ROWS, WG_B)

    def pack_nodal(v):
        out = np.zeros((NCORES, P, NT), dtype=np.float32)
        for c in range(NCORES):
            padded = np.zeros(NLOC_pad, dtype=np.float32)
            padded[:NLOC] = v[c * NLOC:(c + 1) * NLOC]
            out[c] = padded.reshape(NT, P).T
        return out

    dinv2_pk = pack_nodal(dinv2_np)
    outdeg_pk = pack_nodal(outdeg_np.astype(np.float32))
    rdeg_pk = pack_nodal(rdeg_np)

    x_pk = np.zeros((NCORES, NLOC_pad, F), dtype=np.float32)
    for c in range(NCORES):
        x_pk[c, :NLOC] = x[c * NLOC:(c + 1) * NLOC]

    R_np = np.tile(np.arange(P, dtype=np.float16)[None, :], (P, 1))
    id128_np = np.eye(P, dtype=np.float32)
    ones128_np = np.ones((P, 1), dtype=np.float32)
    ones8_np = np.ones((NCORES, 1), dtype=np.float16)
    ones1x128_np = np.ones((1, P), dtype=np.float16)

    # ---------------- device program ----------------
    nc = bacc.Bacc("TRN2", target_bir_lowering=False, debug=False,
                   num_devices=NCORES)

    def din(name, shape, dt):
        return nc.dram_tensor(name, list(shape), dt, kind="ExternalInput").ap()

    x_e = din("x", (NLOC_pad, F), FP32)
    idxA_e = din("idxA", pcA[0]["idxw"].shape, I16)
    drA_e = din("drA", pcA[0]["drw"].shape, FP32)
    wwA_e = din("wwA", pcA[0]["ww"].shape, FP32)
    idxB_e = din("idxB", pcB[0]["idxw"].shape, I16)
    drB_e = din("drB", pcB[0]["drw"].shape, FP32)
    wwB_e = din("wwB", pcB[0]["ww"].shape, FP32)
    dinv2_e = din("dinv2", (P, NT), FP32)
    outdeg_e = din("outdeg", (P, NT), FP32)
    rdeg_e = din("rdeg", (P, NT), FP32)
    R_e = din("Riota", (P, P), FP16)
    id_e = din("id128", (P, P), FP32)
    ones128_e = din("ones128", (P, 1), FP32)
    ones8_e = din("ones8", (NCORES, 1), FP16)
    ones1x128_e = din("ones1x128", (1, P), FP16)
    encw_e = din("enc_w", (F, H), FP32)
    encb_e = din("enc_b", (H, 1), FP32)
    skipw_e = din("skip_w", (F, H), FP32)
    convw_e = din("conv_w", (H, H), FP32)
    convb_e = din("conv_b", (H, 1), FP32)
    decw_e = din("dec_w", (H, OUTF), FP32)
    decb_e = din("dec_b", (OUTF, 1), FP32)
    out_e = nc.dram_tensor("out", [NLOC_pad, OUTF], FP32,
                           kind="ExternalOutput").ap()
    import os as _os
    KDBG = bool(_os.environ.get("KDBG"))
    if KDBG:
        dbg_h0 = nc.dram_tensor("dbg_h0", [P, NT * H], FP32, kind="ExternalOutput").ap()
        dbg_agg = nc.dram_tensor("dbg_agg", [P, NT * H], FP32, kind="ExternalOutput").ap()
        dbg_tu = nc.dram_tensor("dbg_tu", [P, NT * (H + 1)], FP32, kind="ExternalOutput").ap()
        dbg_gmb = nc.dram_tensor("dbg_gmb", [P, H], FP32, kind="ExternalOutput").ap()
        dbg_h1 = nc.dram_tensor("dbg_h1", [P, NT * H], FP32, kind="ExternalOutput").ap()

    aginp = [nc.dram_tensor(f"aginp{l}", [TROWS, P], FP16)
             for l in range(NLAYERS)]
    tableAG = [nc.dram_tensor(f"tableAG{l}", [TOT_ROWS, P], FP16,
                              addr_space="Shared") for l in range(NLAYERS)]

    with tile.TileContext(nc) as tc:
        with (
            tc.tile_pool(name="const", bufs=1) as cpool,
            tc.tile_pool(name="slab", bufs=1) as slab,
            tc.tile_pool(name="io", bufs=3) as io,
            tc.tile_pool(name="mpool", bufs=4) as mpool,
            tc.tile_pool(name="gpool", bufs=2) as gpool,
            tc.tile_pool(name="meta", bufs=2) as meta,
            tc.tile_pool(name="scr", bufs=3) as scr,
            tc.tile_pool(name="ps_big", bufs=2, space="PSUM") as ps_big,
            tc.tile_pool(name="ps_t", bufs=2, space="PSUM") as ps_t,
            tc.tile_pool(name="ps_f", bufs=2, space="PSUM") as ps_f,
            tc.tile_pool(name="ps_s", bufs=1, space="PSUM") as ps_s,
        ):
            def load_const(ext, shape, dt):
                t = cpool.tile(list(shape), dt, name=ext.tensor.name + "_sb")
                nc.sync.dma_start(out=t[:], in_=ext[:, :])
                return t

            R_t = load_const(R_e, (P, P), FP16)
            id_t = load_const(id_e, (P, P), FP32)
            ones128_t = load_const(ones128_e, (P, 1), FP32)
            ones8_t = load_const(ones8_e, (NCORES, 1), FP16)
            ones1x128_t = load_const(ones1x128_e, (1, P), FP16)
            encw_t = load_const(encw_e, (F, H), FP32)
            encb_t = load_const(encb_e, (H, 1), FP32)
            skipw_t = load_const(skipw_e, (F, H), FP32)
            convw_t = load_const(convw_e, (H, H), FP32)
            convb_t = load_const(convb_e, (H, 1), FP32)
            decw_t = load_const(decw_e, (H, OUTF), FP32)
            decb_t = load_const(decb_e, (OUTF, 1), FP32)
            dinv2_t = load_const(dinv2_e, (P, NT), FP32)
            outdeg_t = load_const(outdeg_e, (P, NT), FP32)
            rdeg_t = load_const(rdeg_e, (P, NT), FP32)

            h_sl = slab.tile([P, NT * H], FP32)
            xskip_sl = slab.tile([P, NT * H], FP32)
            agg_sl = slab.tile([P, NT * H], FP32)
            tu_sl = slab.tile([P, NT * (H + 1)], FP32)
            s_sl = slab.tile([P, NT], FP32)
            gmb_sl = slab.tile([P, H], FP32)
            zero16 = cpool.tile([P, P], FP16)
            nc.vector.memset(zero16[:], 0)

            def hsl(t):
                return h_sl[:, t * H:(t + 1) * H]

            def xssl(t):
                return xskip_sl[:, t * H:(t + 1) * H]

            def aggsl(t):
                return agg_sl[:, t * H:(t + 1) * H]

            def tsl(t):
                return tu_sl[:, t * (H + 1):t * (H + 1) + H]

            def usl(t):
                return tu_sl[:, t * (H + 1) + H:(t + 1) * (H + 1)]

            def pt_tile():
                return ps_t.tile([P, P], FP32, space="PSUM", tag="pt", name="pt")

            def pf_tile():
                return ps_f.tile([P, P], FP32, space="PSUM", tag="pf", name="pf")

            # ---------- encoder ----------
            for t in range(NT):
                xt = io.tile([P, F], FP32, tag="xt")
                nc.sync.dma_start(out=xt[:], in_=x_e[t * P:(t + 1) * P, :])
                pxT = pt_tile()
                nc.tensor.transpose(pxT[:], xt[:], id_t[:])
                xT = scr.tile([P, P], FP32, tag="xT")
                nc.vector.tensor_copy(xT[:], pxT[:])

                pf = pf_tile()
                nc.tensor.matmul(pf[0:H, :], lhsT=encw_t[:], rhs=xT[:],
                                 start=True, stop=True)
                hT = scr.tile([H, P], FP32, tag="hT")
                nc.scalar.activation(hT[:], pf[0:H, :], Act.Relu,
                                     bias=encb_t[:, 0:1], scale=1.0)
                pb = pt_tile()
                nc.tensor.transpose(pb[:, 0:H], hT[:], id_t[0:H, 0:H])
                nc.vector.tensor_copy(hsl(t), pb[:, 0:H])

                pf2 = pf_tile()
                nc.tensor.matmul(pf2[0:H, :], lhsT=skipw_t[:], rhs=xT[:],
                                 start=True, stop=True)
                sT = scr.tile([H, P], FP32, tag="hT")
                nc.vector.tensor_copy(sT[:], pf2[0:H, :])
                pb2 = pt_tile()
                nc.tensor.transpose(pb2[:, 0:H], sT[:], id_t[0:H, 0:H])
                nc.vector.tensor_copy(xssl(t), pb2[:, 0:H])

            if KDBG:
                nc.sync.dma_start(out=dbg_h0[:, :], in_=h_sl[:])

            # ---------- layers ----------
            for l in range(NLAYERS):
                # table build
                pcs = ps_s.tile([1, H], FP32, space="PSUM", tag="pcs")
                for t in range(NT):
                    sq = scr.tile([P, H], FP32, tag="sq")
                    nc.vector.tensor_mul(sq[:], hsl(t), hsl(t))
                    nc.vector.tensor_reduce(s_sl[:, t:t + 1], sq[:],
                                            axis=mybir.AxisListType.X,
                                            op=Alu.add)
                    tb = io.tile([P, P], FP16, tag="tb")
                    nc.vector.tensor_copy(tb[:, 0:H], hsl(t))
                    nc.vector.tensor_copy(tb[:, H:H + 1], s_sl[:, t:t + 1])
                    nc.vector.tensor_copy(tb[:, H + 1:P], zero16[:, H + 1:P])
                    nc.sync.dma_start(out=aginp[l][t * P:(t + 1) * P, :],
                                      in_=tb[:])
                    nc.tensor.matmul(pcs[:], lhsT=ones128_t[:], rhs=hsl(t),
                                     start=(t == 0), stop=(t == NT - 1),
                                     skip_group_check=True)
                csum = scr.tile([1, H], FP32, tag="csum")
                nc.scalar.activation(csum[:], pcs[:], Act.Copy, bias=0.0,
                                     scale=1.0 / N)
                csrow = io.tile([1, P], FP16, tag="csrow")
                nc.vector.tensor_copy(csrow[:, 0:H], csum[:])
                nc.vector.tensor_copy(csrow[:, H:P], zero16[0:1, H:P])
                nc.sync.dma_start(out=aginp[l][NLOC_pad:NLOC_pad + 1, :],
                                  in_=csrow[:])
                nc.sync.dma_start(out=aginp[l][NLOC_pad + 1:NLOC_pad + P, :],
                                  in_=zero16[0:P - 1, :])

                nc.gpsimd.collective_compute(
                    "AllGather", Alu.bypass,
                    replica_groups=[list(range(NCORES))],
                    ins=[aginp[l][:, :]],
                    outs=[tableAG[l][:, :]],
                )

                # gmean from the 8 colsum/N rows
                gm8 = scr.tile([NCORES, H], FP16, tag="gm8")
                tview = tableAG[l][:, :].rearrange("(r t) e -> r t e", r=NCORES)
                nc.sync.dma_start(out=gm8[:], in_=tview[:, NLOC_pad, 0:H])
                pgm = ps_s.tile([1, H], FP32, space="PSUM", tag="pcs")
                nc.tensor.matmul(pgm[:], lhsT=ones8_t[:], rhs=gm8[:],
                                 start=True, stop=True)
                gmrow = scr.tile([1, H], FP16, tag="gmrow")
                nc.vector.tensor_copy(gmrow[:], pgm[:])
                pgb = pf_tile()
                nc.tensor.matmul(pgb[:, 0:H], lhsT=ones1x128_t[:], rhs=gmrow[:],
                                 start=True, stop=True)
                nc.vector.tensor_copy(gmb_sl[:], pgb[:, 0:H])

                # ---- gather/segsum passes ----
                def run_pass(sched, idx_e, dr_e, ww_e, WG, FW, out_slab_fn):
                    chunk_win = sched["chunk_win"]
                    cst, csp = sched["chunk_start"], sched["chunk_stop"]
                    state = dict(g=-1, pgrp=None, gw=0)

                    def evac():
                        g, gw, pg = state["g"], state["gw"], state["pgrp"]
                        w0 = g * WG
                        nc.scalar.activation(out_slab_fn(w0, gw),
                                             pg[:, 0:gw * FW], Act.Copy,
                                             bias=0.0, scale=1.0)

                    for (c0, nch, k, g) in sched["calls"]:
                        if g != state["g"]:
                            if state["pgrp"] is not None:
                                evac()
                            state["g"] = g
                            state["gw"] = min(WG, NT - g * WG)
                            state["pgrp"] = ps_big.tile([P, 512], FP32, space="PSUM", tag="pgrp", name="pgrp")
                        pgrp = state["pgrp"]
                        L = nch * P
                        idx_t = meta.tile([P, CALL_MAX_CH * 8], I16, tag="idx")
                        nc.sync.dma_start(out=idx_t[:, 0:nch * 8],
                                          in_=idx_e[:, c0 * 8:(c0 + nch) * 8])
                        dr_t = meta.tile([P, CALL_MAX_CH], FP32, tag="dr")
                        nc.sync.dma_start(out=dr_t[:, 0:nch],
                                          in_=dr_e[:, c0:c0 + nch])
                        w_t = meta.tile([P, CALL_MAX_CH], FP32, tag="wt")
                        nc.sync.dma_start(out=w_t[:, 0:nch],
                                          in_=ww_e[:, c0:c0 + nch])
                        g_t = gpool.tile([P, CALL_MAX_CH * P], FP16, tag="gt")
                        g3 = g_t[:].rearrange("p (c e) -> p c e", c=CALL_MAX_CH)
                        nc.gpsimd.dma_gather(
                            out_ap=g3[:, 0:nch, :],
                            in_ap=tableAG[l][k * CHUNK_ROWS:(k + 1) * CHUNK_ROWS, :],
                            idxs_ap=idx_t[:, 0:nch * 8],
                            num_idxs=L, num_idxs_reg=L, elem_size=P,
                            single_packet=False,
                        )
                        for j in range(nch):
                            cj = c0 + j
                            w = int(chunk_win[cj])
                            wloc = w - g * WG
                            m_t = mpool.tile([P, P], FP16, tag="m")
                            nc.vector.tensor_scalar(
                                out=m_t[:], in0=R_t[:],
                                scalar1=dr_t[:, j:j + 1],
                                scalar2=w_t[:, j:j + 1],
                                op0=Alu.is_equal, op1=Alu.mult)
                            nc.tensor.matmul(
                                pgrp[:, wloc * FW:wloc * FW + FW],
                                lhsT=m_t[:], rhs=g3[:, j, 0:FW],
                                start=bool(cst[cj]), stop=bool(csp[cj]),
                                skip_group_check=True)
                    if state["pgrp"] is not None:
                        evac()

                def aggout(w0, gw):
                    return agg_sl[:, w0 * H:(w0 + gw) * H]

                def tuout(w0, gw):
                    return tu_sl[:, w0 * (H + 1):(w0 + gw) * (H + 1)]

                run_pass(schedA, idxA_e, drA_e, wwA_e, WG_A, H, aggout)
                run_pass(schedB, idxB_e, drB_e, wwB_e, WG_B, H + 1, tuout)
                if KDBG and l == 0:
                    nc.sync.dma_start(out=dbg_agg[:, :], in_=agg_sl[:])
                    nc.sync.dma_start(out=dbg_tu[:, :], in_=tu_sl[:])
                    nc.sync.dma_start(out=dbg_gmb[:, :], in_=gmb_sl[:])

                # ---- dense per-tile update ----
                for t in range(NT):
                    q = scr.tile([P, H], FP32, tag="q")
                    nc.vector.tensor_scalar(out=q[:], in0=hsl(t),
                                            scalar1=dinv2_t[:, t:t + 1],
                                            scalar2=None, op0=Alu.mult)
                    nc.vector.tensor_add(q[:], q[:], aggsl(t))
                    pq = pt_tile()
                    nc.tensor.transpose(pq[0:H, :], q[:], id_t[:])
                    qT = scr.tile([H, P], FP32, tag="qT")
                    nc.vector.tensor_copy(qT[:], pq[0:H, :])
                    pxa = pf_tile()
                    nc.tensor.matmul(pxa[0:H, :], lhsT=convw_t[:], rhs=qT[:],
                                     start=True, stop=True)
                    xaT = scr.tile([H, P], FP32, tag="xaT")
                    nc.scalar.activation(xaT[:], pxa[0:H, :], Act.Relu,
                                         bias=convb_t[:, 0:1], scale=1.0)
                    pxa2 = pt_tile()
                    nc.tensor.transpose(pxa2[:, 0:H], xaT[:], id_t[0:H, 0:H])
                    xagg = scr.tile([P, H], FP32, tag="xagg")
                    nc.vector.tensor_copy(xagg[:], pxa2[:, 0:H])

                    ht = scr.tile([P, H], FP32, tag="ht")
                    nc.vector.tensor_mul(ht[:], hsl(t), tsl(t))
                    hdot = scr.tile([P, 1], FP32, tag="hdot")
                    nc.vector.tensor_reduce(hdot[:], ht[:],
                                            axis=mybir.AxisListType.X,
                                            op=Alu.add)
                    ds = scr.tile([P, 1], FP32, tag="ds")
                    nc.vector.tensor_scalar(out=ds[:], in0=hdot[:],
                                            scalar1=-2.0, scalar2=None,
                                            op0=Alu.mult)
                    nc.vector.tensor_add(ds[:], ds[:], usl(t))
                    os_ = scr.tile([P, 1], FP32, tag="os")
                    nc.vector.tensor_scalar(out=os_[:], in0=s_sl[:, t:t + 1],
                                            scalar1=outdeg_t[:, t:t + 1],
                                            scalar2=None, op0=Alu.mult)
                    nc.vector.tensor_add(ds[:], ds[:], os_[:])
                    gs = scr.tile([P, 1], FP32, tag="gs")
                    nc.scalar.activation(gs[:], ds[:], Act.Tanh,
                                         scale=rdeg_t[:, t:t + 1])

                    df = scr.tile([P, H], FP32, tag="df")
                    nc.vector.tensor_sub(df[:], hsl(t), gmb_sl[:])
                    av = scr.tile([P, H], FP32, tag="av")
                    nc.scalar.activation(av[:], df[:], Act.Abs)
                    cv = scr.tile([P, H], FP32, tag="cv")
                    nc.scalar.activation(cv[:], av[:], Act.Sqrt)
                    nc.vector.tensor_mul(cv[:], cv[:], av[:])   # a^1.5
                    nc.vector.tensor_mul(cv[:], cv[:], av[:])   # a^2.5
                    dq = scr.tile([P, 1], FP32, tag="dq")
                    nc.vector.tensor_reduce(dq[:], cv[:],
                                            axis=mybir.AxisListType.X,
                                            op=Alu.add)
                    gq = scr.tile([P, 1], FP32, tag="gq")
                    nc.scalar.activation(gq[:], dq[:], Act.Tanh)
                    nc.vector.tensor_scalar(out=gq[:], in0=gq[:], scalar1=-1.0,
                                            scalar2=1.0, op0=Alu.mult,
                                            op1=Alu.add)

                    den = scr.tile([P, 1], FP32, tag="den")
                    nc.vector.tensor_add(den[:], gs[:], gq[:])
                    nc.vector.tensor_scalar(out=den[:], in0=den[:],
                                            scalar1=1.0, scalar2=None,
                                            op0=Alu.add)
                    rec = scr.tile([P, 1], FP32, tag="rec")
                    nc.vector.reciprocal(rec[:], den[:])

                    u1 = scr.tile([P, H], FP32, tag="u1")
                    nc.vector.tensor_scalar(out=u1[:], in0=xagg[:],
                                            scalar1=gs[:, 0:1], scalar2=None,
                                            op0=Alu.mult)
                    u2 = scr.tile([P, H], FP32, tag="u2")
                    nc.vector.tensor_scalar(out=u2[:], in0=xssl(t),
                                            scalar1=gq[:, 0:1], scalar2=None,
                                            op0=Alu.mult)
                    nc.vector.tensor_add(u1[:], u1[:], u2[:])
                    nc.vector.tensor_add(u1[:], u1[:], hsl(t))
                    nc.vector.tensor_scalar(out=hsl(t), in0=u1[:],
                                            scalar1=rec[:, 0:1], scalar2=None,
                                            op0=Alu.mult)

                if KDBG and l == 0:
                    nc.sync.dma_start(out=dbg_h1[:, :], in_=h_sl[:])

            # ---------- decoder ----------
            for t in range(NT):
                ph = pt_tile()
                nc.tensor.transpose(ph[0:H, :], hsl(t), id_t[:])
                hT2 = scr.tile([H, P], FP32, tag="qT")
                nc.vector.tensor_copy(hT2[:], ph[0:H, :])
                po = pf_tile()
                nc.tensor.matmul(po[0:OUTF, :], lhsT=decw_t[:], rhs=hT2[:],
                                 start=True, stop=True)
                oT = scr.tile([OUTF, P], FP32, tag="oT")
                nc.vector.tensor_scalar(out=oT[:], in0=po[0:OUTF, :],
                                        scalar1=decb_t[:, 0:1], scalar2=None,
                                        op0=Alu.add)
                pob = pt_tile()
                nc.tensor.transpose(pob[:, 0:OUTF], oT[:], id_t[0:OUTF, 0:OUTF])
                ot = io.tile([P, OUTF], FP32, tag="ot")
                nc.vector.tensor_copy(ot[:], pob[:, 0:OUTF])
                nc.sync.dma_start(out=out_e[t * P:(t + 1) * P, :], in_=ot[:])

    nc.compile()

    in_maps = []
    for c in range(NCORES):
        in_maps.append({
            "x": x_pk[c],
            "idxA": pcA[c]["idxw"], "drA": pcA[c]["drw"], "wwA": pcA[c]["ww"],
            "idxB": pcB[c]["idxw"], "drB": pcB[c]["drw"], "wwB": pcB[c]["ww"],
            "dinv2": dinv2_pk[c], "outdeg": outdeg_pk[c], "rdeg": rdeg_pk[c],
            "Riota": R_np, "id128": id128_np, "ones128": ones128_np,
            "ones8": ones8_np, "ones1x128": ones1x128_np,
            "enc_w": enc_w, "enc_b": enc_b.reshape(H, 1),
            "skip_w": skip_w, "conv_w": conv_w,
            "conv_b": conv_b.reshape(H, 1),
            "dec_w": dec_w, "dec_b": dec_b.reshape(OUTF, 1),
        })

    import os
    if os.environ.get("KSIM"):
        from concourse import bass_interp
        sim = bass_interp.MultiCoreSim(nc, num_cores=NCORES)
        for c in range(NCORES):
            for k, v in in_maps[c].items():
                sim.cores[c].tensor(k)[:] = v
        sim.simulate(check_with_hw=False)
        if KDBG:
            kernel.dbg = [{k: np.array(sim.cores[c].tensor(k)) for k in
                           ("dbg_h0", "dbg_agg", "dbg_tu", "dbg_gmb", "dbg_h1")}
                          for c in range(NCORES)]
        out = np.concatenate([np.array(sim.cores[c].tensor("out"))[:NLOC]
                              for c in range(NCORES)], axis=0)
        return out.astype(np.float32)

    import time as _time
    if os.environ.get("KPERF"):
        _perf_time(nc, in_maps)
    t0 = _time.perf_counter()
    res = run_bass_kernel_spmd(nc, in_maps, list(range(NCORES)))
    t1 = _time.perf_counter()
    globals()["LAST_RUN_S"] = t1 - t0
    if os.environ.get("KTIME"):
        t2 = _time.perf_counter()
        res = run_bass_kernel_spmd(nc, in_maps, list(range(NCORES)))
        t3 = _time.perf_counter()
        globals()["LAST_RUN2_S"] = t3 - t2
        globals()["LAST_EXEC_NS"] = int((t3 - t2) * 1e9)
    out = np.concatenate([res.results[c]["out"][:NLOC]
                          for c in range(NCORES)], axis=0)
    return out.astype(np.float32)


def _perf_time(nc, in_maps):
    """Time warm on-device executions with inputs pre-staged on devices."""
    import time
    import numpy as np
    import jax
    from jax.sharding import Mesh, PartitionSpec, NamedSharding
    from jax.experimental.shard_map import shard_map
    import concourse.mybir as mybir
    from concourse import bass2jax
    from concourse.bass2jax import _bass_exec_p, partition_id_tensor

    bass2jax.install_neuronx_cc_hook()
    n_cores = len(in_maps)
    partition_name = nc.partition_id_tensor.name if nc.partition_id_tensor else None
    in_names, out_names, out_avals, zero_outs = [], [], [], []
    for alloc in nc.m.functions[0].allocations:
        if not isinstance(alloc, mybir.MemoryLocationSet):
            continue
        name = alloc.memorylocations[0].name
        if alloc.kind == "ExternalInput":
            if name != partition_name:
                in_names.append(name)
        elif alloc.kind == "ExternalOutput":
            shape = tuple(alloc.tensor_shape)
            dtype = mybir.dt.np(alloc.dtype)
            out_names.append(name)
            out_avals.append(jax.core.ShapedArray(shape, dtype))
            zero_outs.append(np.zeros(shape, dtype))
    n_params = len(in_names)
    n_outs = len(out_avals)
    in_names.extend(out_names)
    if partition_name is not None:
        in_names.append(partition_name)
    donate = tuple(range(n_params, n_params + n_outs))

    def _body(*args):
        operands = list(args)
        if partition_name is not None:
            operands.append(partition_id_tensor())
        return tuple(_bass_exec_p.bind(
            *operands, out_avals=tuple(out_avals), in_names=tuple(in_names),
            out_names=tuple(out_names), lowering_input_output_aliases=(),
            sim_require_finite=True, sim_require_nnan=True, nc=nc))

    devices = jax.devices()[:n_cores]
    mesh = Mesh(np.asarray(devices), ("core",))
    in_specs = (PartitionSpec("core"),) * (n_params + n_outs)
    out_specs = (PartitionSpec("core"),) * len(out_names)
    sharded = jax.jit(shard_map(_body, mesh=mesh, in_specs=in_specs,
                                out_specs=out_specs, check_rep=False),
                      donate_argnums=donate, keep_unused=True)
    sh = NamedSharding(mesh, PartitionSpec("core"))
    concat_in = [
        jax.device_put(np.concatenate(
            [np.asarray(in_maps[c][in_names[i]]) for c in range(n_cores)], axis=0), sh)
        for i in range(n_params)
    ]
    times = []
    zput = []
    for it in range(4):
        tz = time.perf_counter()
        cz = [jax.device_put(np.zeros((n_cores * z.shape[0], *z.shape[1:]), z.dtype), sh)
              for z in zero_outs]
        jax.block_until_ready(cz)
        t0 = time.perf_counter()
        outs = sharded(*concat_in, *cz)
        jax.block_until_ready(outs)
        t1 = time.perf_counter()
        zput.append(t0 - tz)
        times.append(t1 - t0)
    print("KPERF zeros-put s:", ["%.4f" % t for t in zput])
    print("KPERF exec s:", ["%.4f" % t for t in times])
    globals()["LAST_EXEC_NS"] = int(min(times[1:]) * 1e9)


# revision 17
# speedup vs baseline: 49.1711x; 1.0015x over previous
"""DualGate GNN forward on 8 Trainium2 NeuronCores (Bass/Tile).

Strategy:
  - Nodes sharded across 8 cores (N/8 rows each), weights replicated.
  - Per layer, each core builds a fp16 "table" row-block [TROWS, 128] =
    (h | s=||h||^2 | pad) plus a colsum/N row, AllGathered to a replicated
    table [8*TROWS, 128] in DRAM.
  - Pass A (edges partitioned by DEST core, windows of 128 dest nodes):
    dma_gather of h_src rows + a one-hot-weighted selection matrix M
    (M[e,d] = norm_e * (dest_rel_e == d)) -> PE matmul segmented-sum into
    PSUM window tiles -> agg.  The GCN conv_w multiply is deferred to a
    dense per-node matmul (aggregation is linear).
  - Pass B (edges partitioned by SRC core): same machinery computes
    t_j = sum h_i and u_j = sum s_i over out-edges; then
    ds_j = outdeg_j*s_j - 2 h_j.t_j + u_j  (= sum_out ||h_j - h_i||^2).
  - Gates gs/gq and the node update are dense per-node-tile ops.

Self-contained: derives all sizes from input shapes; no file reads.
"""

import math
import numpy as np

NCORES = 8
P = 128
WG_A = 8    # windows per PSUM group, pass A (8*64 = 512 fp32 = 1 bank)
WG_B = 7    # pass B (7*65 = 455 <= 512)
CALL_MAX_CH = 48  # max chunks (of 128 edges) per dma_gather call
NLAYERS = 2


def _build_pass(dest, src, wgt, trow_of, NLOC, NT, NCHK, CHUNK_ROWS, WG):
    """Per-core slot arrays + a core-equalized static schedule."""
    core = dest // NLOC
    dloc = dest % NLOC
    win = dloc // P
    drel = (dloc % P).astype(np.float32)
    trow = trow_of(src)
    schk = (trow // CHUNK_ROWS).astype(np.int64)
    rel_idx = (trow % CHUNK_ROWS).astype(np.int16)

    NG = math.ceil(NT / WG)

    seg_rank_tab = np.full((NT, NCHK), -1, dtype=np.int64)
    seg_list = []
    for g in range(NG):
        for k in range(NCHK):
            for w in range(g * WG, min((g + 1) * WG, NT)):
                seg_rank_tab[w, k] = len(seg_list)
                seg_list.append((w, k))
    nseg = len(seg_list)

    seg_of_edge = seg_rank_tab[win, schk]
    cnt = np.zeros((NCORES, nseg), dtype=np.int64)
    np.add.at(cnt, (core, seg_of_edge), 1)
    eq_ch = np.ceil(cnt / P).astype(np.int64).max(axis=0)
    # every window needs >= 1 chunk so its PSUM slice gets written
    for w in range(NT):
        if sum(eq_ch[seg_rank_tab[w, k]] for k in range(NCHK)) == 0:
            eq_ch[seg_rank_tab[w, 0]] = 1

    seg_c0 = np.zeros(nseg + 1, dtype=np.int64)
    np.cumsum(eq_ch, out=seg_c0[1:])
    C = int(seg_c0[-1])
    S = C * P

    chunk_win = np.zeros(C, dtype=np.int64)
    for si, (w, k) in enumerate(seg_list):
        chunk_win[seg_c0[si]:seg_c0[si + 1]] = w
    # start/stop at GROUP granularity: start=True zeroes the whole 2KB PSUM
    # bank, so only the first matmul of a group may set it.
    chunk_start = np.zeros(C, dtype=bool)
    chunk_stop = np.zeros(C, dtype=bool)
    for g in range(NG):
        w_lo = g * WG
        w_hi = min((g + 1) * WG, NT) - 1
        gc0 = int(seg_c0[seg_rank_tab[w_lo, 0]])
        gc1 = int(seg_c0[seg_rank_tab[w_hi, NCHK - 1] + 1])
        chunk_start[gc0] = True
        chunk_stop[gc1 - 1] = True

    calls = []
    for g in range(NG):
        ws = list(range(g * WG, min((g + 1) * WG, NT)))
        for k in range(NCHK):
            c0 = int(seg_c0[seg_rank_tab[ws[0], k]])
            c1 = int(seg_c0[seg_rank_tab[ws[-1], k] + 1])
            c = c0
            while c < c1:
                nch = min(CALL_MAX_CH, c1 - c)
                calls.append((c, nch, k, g))
                c += nch

    per_core = []
    for cidx in range(NCORES):
        m = core == cidx
        er = np.nonzero(m)[0]
        order = np.argsort(seg_of_edge[er], kind="stable")
        er = er[order]
        segs = seg_of_edge[er]
        first = np.ones(len(er), dtype=bool)
        if len(er) > 1:
            first[1:] = segs[1:] != segs[:-1]
        startpos = np.nonzero(first)[0]
        runlen = np.diff(np.append(startpos, len(er)))
        within = np.arange(len(er)) - np.repeat(startpos, runlen)
        slot = seg_c0[segs] * P + within

        idx_slots = np.zeros(S, dtype=np.int16)
        dr_slots = np.zeros(S, dtype=np.float32)
        w_slots = np.zeros(S, dtype=np.float32)
        idx_slots[slot] = rel_idx[er]
        dr_slots[slot] = drel[er]
        w_slots[slot] = wgt[er]

        base = idx_slots.reshape(S // 16, 16).T      # [16, S/16]
        idxw = np.ascontiguousarray(np.tile(base, (8, 1)))
        drw = dr_slots.reshape(C, P).T               # [128, C]
        ww = w_slots.reshape(C, P).T
        # host-built selection matrices: M[p, c*128+d] = (drw[p,c]==d)*ww[p,c]
        m_all = ((drw[:, :, None] == np.arange(P, dtype=np.float32))
                 .astype(np.float16) * ww[:, :, None].astype(np.float16))
        m_all = np.ascontiguousarray(m_all.reshape(P, C * P))
        per_core.append(dict(idxw=idxw, m=m_all))

    sched = dict(C=C, calls=calls, chunk_win=chunk_win,
                 chunk_start=chunk_start, chunk_stop=chunk_stop)
    return sched, per_core


def kernel(x, edge_index, enc_w, enc_b, skip_w, conv_w, conv_b, dec_w, dec_b):
    import jax  # noqa: F401  (axon boot side effect)
    import concourse.bass as bass  # noqa: F401
    import concourse.mybir as mybir
    import concourse.tile as tile
    from concourse import bacc
    from concourse.bass_utils import run_bass_kernel_spmd

    FP16, FP32, I16 = mybir.dt.float16, mybir.dt.float32, mybir.dt.int16
    Alu = mybir.AluOpType
    Act = mybir.ActivationFunctionType

    x = np.asarray(x, dtype=np.float32)
    edge_index = np.asarray(edge_index)
    enc_w = np.asarray(enc_w, dtype=np.float32)
    enc_b = np.asarray(enc_b, dtype=np.float32)
    skip_w = np.asarray(skip_w, dtype=np.float32)
    conv_w = np.asarray(conv_w, dtype=np.float32)
    conv_b = np.asarray(conv_b, dtype=np.float32)
    dec_w = np.asarray(dec_w, dtype=np.float32)
    dec_b = np.asarray(dec_b, dtype=np.float32)

    N, F = x.shape
    H = enc_w.shape[1]
    OUTF = dec_w.shape[1]
    assert N % NCORES == 0 and F == 128 and H == 64
    NLOC = N // NCORES
    NT = math.ceil(NLOC / P)
    NLOC_pad = NT * P
    TROWS = NLOC_pad + P
    NCHK = 4
    CHUNK_ROWS = TROWS * NCORES // NCHK
    assert CHUNK_ROWS <= 32767 and TROWS * NCORES % NCHK == 0
    NG_A = math.ceil(NT / WG_A)  # noqa: F841
    TOT_ROWS = TROWS * NCORES

    row = np.asarray(edge_index[0], dtype=np.int64)
    col = np.asarray(edge_index[1], dtype=np.int64)

    deg = np.bincount(col, minlength=N).astype(np.float64) + 1.0
    dinv = 1.0 / np.sqrt(deg)
    norm = (dinv[row] * dinv[col]).astype(np.float32)
    outdeg_np = np.bincount(row, minlength=N).astype(np.float64)
    rdeg_np = (1.0 / (outdeg_np + 1e-10)).astype(np.float32)
    dinv2_np = (dinv * dinv).astype(np.float32)

    def trow_of(n):
        return (n // NLOC) * TROWS + (n % NLOC)

    schedA, pcA = _build_pass(col, row, norm, trow_of, NLOC, NT, NCHK,
                              CHUNK_ROWS, WG_A)
    schedB, pcB = _build_pass(row, col, np.ones(len(row), np.float32), trow_of,
                              NLOC, NT, NCHK, CHUNK_ROWS, WG_B)

    def pack_nodal(v):
        out = np.zeros((NCORES, P, NT), dtype=np.float32)
        for c in range(NCORES):
            padded = np.zeros(NLOC_pad, dtype=np.float32)
            padded[:NLOC] = v[c * NLOC:(c + 1) * NLOC]
            out[c] = padded.reshape(NT, P).T
        return out

    dinv2_pk = pack_nodal(dinv2_np)
    outdeg_pk = pack_nodal(outdeg_np.astype(np.float32))
    rdeg_pk = pack_nodal(rdeg_np)

    x_pk = np.zeros((NCORES, NLOC_pad, F), dtype=np.float32)
    for c in range(NCORES):
        x_pk[c, :NLOC] = x[c * NLOC:(c + 1) * NLOC]

    R_np = np.tile(np.arange(P, dtype=np.float16)[None, :], (P, 1))
    id128_np = np.eye(P, dtype=np.float32)
    ones128_np = np.ones((P, 1), dtype=np.float32)
    ones8_np = np.ones((NCORES, 1), dtype=np.float16)
    ones1x128_np = np.ones((1, P), dtype=np.float16)

    # ---------------- device program ----------------
    nc = bacc.Bacc("TRN2", target_bir_lowering=False, debug=False,
                   num_devices=NCORES)

    def din(name, shape, dt):
        return nc.dram_tensor(name, list(shape), dt, kind="ExternalInput").ap()

    x_e = din("x", (NLOC_pad, F), FP32)
    idxA_e = din("idxA", pcA[0]["idxw"].shape, I16)
    mA_e = din("mA", pcA[0]["m"].shape, FP16)
    idxB_e = din("idxB", pcB[0]["idxw"].shape, I16)
    mB_e = din("mB", pcB[0]["m"].shape, FP16)
    dinv2_e = din("dinv2", (P, NT), FP32)
    outdeg_e = din("outdeg", (P, NT), FP32)
    rdeg_e = din("rdeg", (P, NT), FP32)
    R_e = din("Riota", (P, P), FP16)
    id_e = din("id128", (P, P), FP32)
    ones128_e = din("ones128", (P, 1), FP32)
    ones8_e = din("ones8", (NCORES, 1), FP16)
    ones1x128_e = din("ones1x128", (1, P), FP16)
    encw_e = din("enc_w", (F, H), FP32)
    encb_e = din("enc_b", (H, 1), FP32)
    skipw_e = din("skip_w", (F, H), FP32)
    convw_e = din("conv_w", (H, H), FP32)
    convb_e = din("conv_b", (H, 1), FP32)
    decw_e = din("dec_w", (H, OUTF), FP32)
    decb_e = din("dec_b", (OUTF, 1), FP32)
    out_e = nc.dram_tensor("out", [NLOC_pad, OUTF], FP32,
                           kind="ExternalOutput").ap()
    import os as _os
    KDBG = bool(_os.environ.get("KDBG"))
    if KDBG:
        dbg_h0 = nc.dram_tensor("dbg_h0", [P, NT * H], FP32, kind="ExternalOutput").ap()
        dbg_agg = nc.dram_tensor("dbg_agg", [P, NT * H], FP32, kind="ExternalOutput").ap()
        dbg_tu = nc.dram_tensor("dbg_tu", [P, NT * (H + 1)], FP32, kind="ExternalOutput").ap()
        dbg_gmb = nc.dram_tensor("dbg_gmb", [P, H], FP32, kind="ExternalOutput").ap()
        dbg_h1 = nc.dram_tensor("dbg_h1", [P, NT * H], FP32, kind="ExternalOutput").ap()

    aginp = [nc.dram_tensor(f"aginp{l}", [TROWS, P], FP16)
             for l in range(NLAYERS)]
    tableAG = [nc.dram_tensor(f"tableAG{l}", [TOT_ROWS, P], FP16,
                              addr_space="Shared") for l in range(NLAYERS)]

    with tile.TileContext(nc) as tc:
        with (
            tc.tile_pool(name="const", bufs=1) as cpool,
            tc.tile_pool(name="slab", bufs=1) as slab,
            tc.tile_pool(name="io", bufs=3) as io,
            tc.tile_pool(name="mpool", bufs=4) as mpool,
            tc.tile_pool(name="gpool", bufs=2) as gpool,
            tc.tile_pool(name="meta", bufs=2) as meta,
            tc.tile_pool(name="scr", bufs=3) as scr,
            tc.tile_pool(name="ps_big", bufs=2, space="PSUM") as ps_big,
            tc.tile_pool(name="ps_t", bufs=2, space="PSUM") as ps_t,
            tc.tile_pool(name="ps_f", bufs=2, space="PSUM") as ps_f,
            tc.tile_pool(name="ps_s", bufs=1, space="PSUM") as ps_s,
        ):
            def load_const(ext, shape, dt):
                t = cpool.tile(list(shape), dt, name=ext.tensor.name + "_sb")
                nc.sync.dma_start(out=t[:], in_=ext[:, :])
                return t

            R_t = load_const(R_e, (P, P), FP16)
            id_t = load_const(id_e, (P, P), FP32)
            ones128_t = load_const(ones128_e, (P, 1), FP32)
            ones8_t = load_const(ones8_e, (NCORES, 1), FP16)
            ones1x128_t = load_const(ones1x128_e, (1, P), FP16)
            encw_t = load_const(encw_e, (F, H), FP32)
            encb_t = load_const(encb_e, (H, 1), FP32)
            skipw_t = load_const(skipw_e, (F, H), FP32)
            convw_t = load_const(convw_e, (H, H), FP32)
            convb_t = load_const(convb_e, (H, 1), FP32)
            decw_t = load_const(decw_e, (H, OUTF), FP32)
            decb_t = load_const(decb_e, (OUTF, 1), FP32)
            dinv2_t = load_const(dinv2_e, (P, NT), FP32)
            outdeg_t = load_const(outdeg_e, (P, NT), FP32)
            rdeg_t = load_const(rdeg_e, (P, NT), FP32)

            h_sl = slab.tile([P, NT * H], FP32)
            xskip_sl = slab.tile([P, NT * H], FP32)
            agg_sl = slab.tile([P, NT * H], FP32)
            tu_sl = slab.tile([P, NT * (H + 1)], FP32)
            s_sl = slab.tile([P, NT], FP32)
            gmb_sl = slab.tile([P, H], FP32)
            zero16 = cpool.tile([P, P], FP16)
            nc.vector.memset(zero16[:], 0)

            def hsl(t):
                return h_sl[:, t * H:(t + 1) * H]

            def xssl(t):
                return xskip_sl[:, t * H:(t + 1) * H]

            def aggsl(t):
                return agg_sl[:, t * H:(t + 1) * H]

            def tsl(t):
                return tu_sl[:, t * (H + 1):t * (H + 1) + H]

            def usl(t):
                return tu_sl[:, t * (H + 1) + H:(t + 1) * (H + 1)]

            def pt_tile():
                return ps_t.tile([P, P], FP32, space="PSUM", tag="pt", name="pt")

            def pf_tile():
                return ps_f.tile([P, P], FP32, space="PSUM", tag="pf", name="pf")

            # ---------- encoder ----------
            for t in range(NT):
                xt = io.tile([P, F], FP32, tag="xt")
                nc.sync.dma_start(out=xt[:], in_=x_e[t * P:(t + 1) * P, :])
                pxT = pt_tile()
                nc.tensor.transpose(pxT[:], xt[:], id_t[:])
                xT = scr.tile([P, P], FP32, tag="xT")
                nc.vector.tensor_copy(xT[:], pxT[:])

                pf = pf_tile()
                nc.tensor.matmul(pf[0:H, :], lhsT=encw_t[:], rhs=xT[:],
                                 start=True, stop=True)
                hT = scr.tile([H, P], FP32, tag="hT")
                nc.scalar.activation(hT[:], pf[0:H, :], Act.Relu,
                                     bias=encb_t[:, 0:1], scale=1.0)
                pb = pt_tile()
                nc.tensor.transpose(pb[:, 0:H], hT[:], id_t[0:H, 0:H])
                nc.vector.tensor_copy(hsl(t), pb[:, 0:H])

                pf2 = pf_tile()
                nc.tensor.matmul(pf2[0:H, :], lhsT=skipw_t[:], rhs=xT[:],
                                 start=True, stop=True)
                sT = scr.tile([H, P], FP32, tag="hT")
                nc.vector.tensor_copy(sT[:], pf2[0:H, :])
                pb2 = pt_tile()
                nc.tensor.transpose(pb2[:, 0:H], sT[:], id_t[0:H, 0:H])
                nc.vector.tensor_copy(xssl(t), pb2[:, 0:H])

            if KDBG:
                nc.sync.dma_start(out=dbg_h0[:, :], in_=h_sl[:])

            # ---------- layers ----------
            _rep = int(_os.environ.get("KREPEAT", "1"))
            for _li in range(NLAYERS * _rep):
                l = _li % NLAYERS
                # table build
                pcs = ps_s.tile([1, H], FP32, space="PSUM", tag="pcs")
                for t in range(NT):
                    sq = scr.tile([P, H], FP32, tag="sq")
                    nc.vector.tensor_mul(sq[:], hsl(t), hsl(t))
                    nc.vector.tensor_reduce(s_sl[:, t:t + 1], sq[:],
                                            axis=mybir.AxisListType.X,
                                            op=Alu.add)
                    tb = io.tile([P, P], FP16, tag="tb")
                    nc.vector.tensor_copy(tb[:, 0:H], hsl(t))
                    nc.vector.tensor_copy(tb[:, H:H + 1], s_sl[:, t:t + 1])
                    nc.vector.tensor_copy(tb[:, H + 1:P], zero16[:, H + 1:P])
                    nc.sync.dma_start(out=aginp[l][t * P:(t + 1) * P, :],
                                      in_=tb[:])
                    nc.tensor.matmul(pcs[:], lhsT=ones128_t[:], rhs=hsl(t),
                                     start=(t == 0), stop=(t == NT - 1),
                                     skip_group_check=True)
                csum = scr.tile([1, H], FP32, tag="csum")
                nc.scalar.activation(csum[:], pcs[:], Act.Copy, bias=0.0,
                                     scale=1.0 / N)
                csrow = io.tile([1, P], FP16, tag="csrow")
                nc.vector.tensor_copy(csrow[:, 0:H], csum[:])
                nc.vector.tensor_copy(csrow[:, H:P], zero16[0:1, H:P])
                nc.sync.dma_start(out=aginp[l][NLOC_pad:NLOC_pad + 1, :],
                                  in_=csrow[:])
                nc.sync.dma_start(out=aginp[l][NLOC_pad + 1:NLOC_pad + P, :],
                                  in_=zero16[0:P - 1, :])

                nc.gpsimd.collective_compute(
                    "AllGather", Alu.bypass,
                    replica_groups=[list(range(NCORES))],
                    ins=[aginp[l][:, :]],
                    outs=[tableAG[l][:, :]],
                )

                # gmean from the 8 colsum/N rows
                gm8 = scr.tile([NCORES, H], FP16, tag="gm8")
                tview = tableAG[l][:, :].rearrange("(r t) e -> r t e", r=NCORES)
                nc.sync.dma_start(out=gm8[:], in_=tview[:, NLOC_pad, 0:H])
                pgm = ps_s.tile([1, H], FP32, space="PSUM", tag="pcs")
                nc.tensor.matmul(pgm[:], lhsT=ones8_t[:], rhs=gm8[:],
                                 start=True, stop=True)
                gmrow = scr.tile([1, H], FP16, tag="gmrow")
                nc.vector.tensor_copy(gmrow[:], pgm[:])
                pgb = pf_tile()
                nc.tensor.matmul(pgb[:, 0:H], lhsT=ones1x128_t[:], rhs=gmrow[:],
                                 start=True, stop=True)
                nc.vector.tensor_copy(gmb_sl[:], pgb[:, 0:H])

                # ---- gather/segsum passes ----
                def run_pass(sched, idx_e, m_e, WG, FW, out_slab_fn):
                    chunk_win = sched["chunk_win"]
                    cst, csp = sched["chunk_start"], sched["chunk_stop"]
                    state = dict(g=-1, pgrp=None, gw=0)

                    def evac():
                        if _os.environ.get("KGONLY"):
                            return
                        g, gw, pg = state["g"], state["gw"], state["pgrp"]
                        w0 = g * WG
                        nc.scalar.activation(out_slab_fn(w0, gw),
                                             pg[:, 0:gw * FW], Act.Copy,
                                             bias=0.0, scale=1.0)

                    for (c0, nch, k, g) in sched["calls"]:
                        if g != state["g"]:
                            if state["pgrp"] is not None:
                                evac()
                            state["g"] = g
                            state["gw"] = min(WG, NT - g * WG)
                            state["pgrp"] = ps_big.tile([P, 512], FP32, space="PSUM", tag="pgrp", name="pgrp")
                        pgrp = state["pgrp"]
                        L = nch * P
                        idx_t = meta.tile([P, CALL_MAX_CH * 8], I16, tag="idx")
                        nc.sync.dma_start(out=idx_t[:, 0:nch * 8],
                                          in_=idx_e[:, c0 * 8:(c0 + nch) * 8])
                        m_all_t = meta.tile([P, CALL_MAX_CH * P], FP16,
                                            tag="mld")
                        nc.sync.dma_start(out=m_all_t[:, 0:nch * P],
                                          in_=m_e[:, c0 * P:(c0 + nch) * P])
                        g_t = gpool.tile([P, CALL_MAX_CH * P], FP16, tag="gt")
                        g3 = g_t[:].rearrange("p (c e) -> p c e", c=CALL_MAX_CH)
                        nc.gpsimd.dma_gather(
                            out_ap=g3[:, 0:nch, :],
                            in_ap=tableAG[l][k * CHUNK_ROWS:(k + 1) * CHUNK_ROWS, :],
                            idxs_ap=idx_t[:, 0:nch * 8],
                            num_idxs=L, num_idxs_reg=L, elem_size=P,
                            single_packet=False,
                        )
                        if _os.environ.get("KGONLY"):
                            continue
                        for j in range(nch):
                            cj = c0 + j
                            w = int(chunk_win[cj])
                            wloc = w - g * WG
                            nc.tensor.matmul(
                                pgrp[:, wloc * FW:wloc * FW + FW],
                                lhsT=m_all_t[:, j * P:(j + 1) * P],
                                rhs=g3[:, j, 0:FW],
                                start=bool(cst[cj]), stop=bool(csp[cj]),
                                skip_group_check=True)
                    if state["pgrp"] is not None:
                        evac()

                def aggout(w0, gw):
                    return agg_sl[:, w0 * H:(w0 + gw) * H]

                def tuout(w0, gw):
                    return tu_sl[:, w0 * (H + 1):(w0 + gw) * (H + 1)]

                if _os.environ.get("KNOPASS") or _os.environ.get("KGONLY"):
                    nc.vector.memset(agg_sl[:], 0)
                    nc.vector.memset(tu_sl[:], 0)
                if not _os.environ.get("KNOPASS"):
                    run_pass(schedA, idxA_e, mA_e, WG_A, H, aggout)
                    run_pass(schedB, idxB_e, mB_e, WG_B, H + 1, tuout)
                if KDBG and l == 0:
                    nc.sync.dma_start(out=dbg_agg[:, :], in_=agg_sl[:])
                    nc.sync.dma_start(out=dbg_tu[:, :], in_=tu_sl[:])
                    nc.sync.dma_start(out=dbg_gmb[:, :], in_=gmb_sl[:])

                # ---- dense per-tile update ----
                for t in range(NT if not _os.environ.get("KNODENSE") else 0):
                    q = scr.tile([P, H], FP32, tag="q")
                    nc.vector.tensor_scalar(out=q[:], in0=hsl(t),
                                            scalar1=dinv2_t[:, t:t + 1],
                                            scalar2=None, op0=Alu.mult)
                    nc.vector.tensor_add(q[:], q[:], aggsl(t))
                    pq = pt_tile()
                    nc.tensor.transpose(pq[0:H, :], q[:], id_t[:])
                    qT = scr.tile([H, P], FP32, tag="qT")
                    nc.vector.tensor_copy(qT[:], pq[0:H, :])
                    pxa = pf_tile()
                    nc.tensor.matmul(pxa[0:H, :], lhsT=convw_t[:], rhs=qT[:],
                                     start=True, stop=True)
                    xaT = scr.tile([H, P], FP32, tag="xaT")
                    nc.scalar.activation(xaT[:], pxa[0:H, :], Act.Relu,
                                         bias=convb_t[:, 0:1], scale=1.0)
                    pxa2 = pt_tile()
                    nc.tensor.transpose(pxa2[:, 0:H], xaT[:], id_t[0:H, 0:H])
                    xagg = scr.tile([P, H], FP32, tag="xagg")
                    nc.vector.tensor_copy(xagg[:], pxa2[:, 0:H])

                    ht = scr.tile([P, H], FP32, tag="ht")
                    nc.vector.tensor_mul(ht[:], hsl(t), tsl(t))
                    hdot = scr.tile([P, 1], FP32, tag="hdot")
                    nc.vector.tensor_reduce(hdot[:], ht[:],
                                            axis=mybir.AxisListType.X,
                                            op=Alu.add)
                    ds = scr.tile([P, 1], FP32, tag="ds")
                    nc.vector.tensor_scalar(out=ds[:], in0=hdot[:],
                                            scalar1=-2.0, scalar2=None,
                                            op0=Alu.mult)
                    nc.vector.tensor_add(ds[:], ds[:], usl(t))
                    os_ = scr.tile([P, 1], FP32, tag="os")
                    nc.vector.tensor_scalar(out=os_[:], in0=s_sl[:, t:t + 1],
                                            scalar1=outdeg_t[:, t:t + 1],
                                            scalar2=None, op0=Alu.mult)
                    nc.vector.tensor_add(ds[:], ds[:], os_[:])
                    gs = scr.tile([P, 1], FP32, tag="gs")
                    nc.scalar.activation(gs[:], ds[:], Act.Tanh,
                                         scale=rdeg_t[:, t:t + 1])

                    df = scr.tile([P, H], FP32, tag="df")
                    nc.vector.tensor_sub(df[:], hsl(t), gmb_sl[:])
                    av = scr.tile([P, H], FP32, tag="av")
                    nc.scalar.activation(av[:], df[:], Act.Abs)
                    cv = scr.tile([P, H], FP32, tag="cv")
                    nc.scalar.activation(cv[:], av[:], Act.Sqrt)
                    nc.vector.tensor_mul(cv[:], cv[:], av[:])   # a^1.5
                    nc.vector.tensor_mul(cv[:], cv[:], av[:])   # a^2.5
                    dq = scr.tile([P, 1], FP32, tag="dq")
                    nc.vector.tensor_reduce(dq[:], cv[:],
                                            axis=mybir.AxisListType.X,
                                            op=Alu.add)
                    gq = scr.tile([P, 1], FP32, tag="gq")
                    nc.scalar.activation(gq[:], dq[:], Act.Tanh)
                    nc.vector.tensor_scalar(out=gq[:], in0=gq[:], scalar1=-1.0,
                                            scalar2=1.0, op0=Alu.mult,
                                            op1=Alu.add)

                    den = scr.tile([P, 1], FP32, tag="den")
                    nc.vector.tensor_add(den[:], gs[:], gq[:])
                    nc.vector.tensor_scalar(out=den[:], in0=den[:],
                                            scalar1=1.0, scalar2=None,
                                            op0=Alu.add)
                    rec = scr.tile([P, 1], FP32, tag="rec")
                    nc.vector.reciprocal(rec[:], den[:])

                    u1 = scr.tile([P, H], FP32, tag="u1")
                    nc.vector.tensor_scalar(out=u1[:], in0=xagg[:],
                                            scalar1=gs[:, 0:1], scalar2=None,
                                            op0=Alu.mult)
                    u2 = scr.tile([P, H], FP32, tag="u2")
                    nc.vector.tensor_scalar(out=u2[:], in0=xssl(t),
                                            scalar1=gq[:, 0:1], scalar2=None,
                                            op0=Alu.mult)
                    nc.vector.tensor_add(u1[:], u1[:], u2[:])
                    nc.vector.tensor_add(u1[:], u1[:], hsl(t))
                    nc.vector.tensor_scalar(out=hsl(t), in0=u1[:],
                                            scalar1=rec[:, 0:1], scalar2=None,
                                            op0=Alu.mult)

                if KDBG and l == 0:
                    nc.sync.dma_start(out=dbg_h1[:, :], in_=h_sl[:])

            # ---------- decoder ----------
            for t in range(NT):
                ph = pt_tile()
                nc.tensor.transpose(ph[0:H, :], hsl(t), id_t[:])
                hT2 = scr.tile([H, P], FP32, tag="qT")
                nc.vector.tensor_copy(hT2[:], ph[0:H, :])
                po = pf_tile()
                nc.tensor.matmul(po[0:OUTF, :], lhsT=decw_t[:], rhs=hT2[:],
                                 start=True, stop=True)
                oT = scr.tile([OUTF, P], FP32, tag="oT")
                nc.vector.tensor_scalar(out=oT[:], in0=po[0:OUTF, :],
                                        scalar1=decb_t[:, 0:1], scalar2=None,
                                        op0=Alu.add)
                pob = pt_tile()
                nc.tensor.transpose(pob[:, 0:OUTF], oT[:], id_t[0:OUTF, 0:OUTF])
                ot = io.tile([P, OUTF], FP32, tag="ot")
                nc.vector.tensor_copy(ot[:], pob[:, 0:OUTF])
                nc.sync.dma_start(out=out_e[t * P:(t + 1) * P, :], in_=ot[:])

    nc.compile()

    in_maps = []
    for c in range(NCORES):
        in_maps.append({
            "x": x_pk[c],
            "idxA": pcA[c]["idxw"], "mA": pcA[c]["m"],
            "idxB": pcB[c]["idxw"], "mB": pcB[c]["m"],
            "dinv2": dinv2_pk[c], "outdeg": outdeg_pk[c], "rdeg": rdeg_pk[c],
            "Riota": R_np, "id128": id128_np, "ones128": ones128_np,
            "ones8": ones8_np, "ones1x128": ones1x128_np,
            "enc_w": enc_w, "enc_b": enc_b.reshape(H, 1),
            "skip_w": skip_w, "conv_w": conv_w,
            "conv_b": conv_b.reshape(H, 1),
            "dec_w": dec_w, "dec_b": dec_b.reshape(OUTF, 1),
        })

    import os
    if os.environ.get("KSIM"):
        from concourse import bass_interp
        sim = bass_interp.MultiCoreSim(nc, num_cores=NCORES)
        for c in range(NCORES):
            for k, v in in_maps[c].items():
                sim.cores[c].tensor(k)[:] = v
        sim.simulate(check_with_hw=False)
        if KDBG:
            kernel.dbg = [{k: np.array(sim.cores[c].tensor(k)) for k in
                           ("dbg_h0", "dbg_agg", "dbg_tu", "dbg_gmb", "dbg_h1")}
                          for c in range(NCORES)]
        out = np.concatenate([np.array(sim.cores[c].tensor("out"))[:NLOC]
                              for c in range(NCORES)], axis=0)
        return out.astype(np.float32)

    import time as _time
    if os.environ.get("KPERF"):
        _perf_time(nc, in_maps)
    t0 = _time.perf_counter()
    res = run_bass_kernel_spmd(nc, in_maps, list(range(NCORES)))
    t1 = _time.perf_counter()
    globals()["LAST_RUN_S"] = t1 - t0
    if os.environ.get("KTIME"):
        t2 = _time.perf_counter()
        res = run_bass_kernel_spmd(nc, in_maps, list(range(NCORES)))
        t3 = _time.perf_counter()
        globals()["LAST_RUN2_S"] = t3 - t2
        globals()["LAST_EXEC_NS"] = int((t3 - t2) * 1e9)
    out = np.concatenate([res.results[c]["out"][:NLOC]
                          for c in range(NCORES)], axis=0)
    return out.astype(np.float32)


def _perf_time(nc, in_maps):
    """Time warm on-device executions with inputs pre-staged on devices."""
    import time
    import numpy as np
    import jax
    from jax.sharding import Mesh, PartitionSpec, NamedSharding
    from jax.experimental.shard_map import shard_map
    import concourse.mybir as mybir
    from concourse import bass2jax
    from concourse.bass2jax import _bass_exec_p, partition_id_tensor

    bass2jax.install_neuronx_cc_hook()
    n_cores = len(in_maps)
    partition_name = nc.partition_id_tensor.name if nc.partition_id_tensor else None
    in_names, out_names, out_avals, zero_outs = [], [], [], []
    for alloc in nc.m.functions[0].allocations:
        if not isinstance(alloc, mybir.MemoryLocationSet):
            continue
        name = alloc.memorylocations[0].name
        if alloc.kind == "ExternalInput":
            if name != partition_name:
                in_names.append(name)
        elif alloc.kind == "ExternalOutput":
            shape = tuple(alloc.tensor_shape)
            dtype = mybir.dt.np(alloc.dtype)
            out_names.append(name)
            out_avals.append(jax.core.ShapedArray(shape, dtype))
            zero_outs.append(np.zeros(shape, dtype))
    n_params = len(in_names)
    n_outs = len(out_avals)
    in_names.extend(out_names)
    if partition_name is not None:
        in_names.append(partition_name)
    donate = tuple(range(n_params, n_params + n_outs))

    def _body(*args):
        operands = list(args)
        if partition_name is not None:
            operands.append(partition_id_tensor())
        return tuple(_bass_exec_p.bind(
            *operands, out_avals=tuple(out_avals), in_names=tuple(in_names),
            out_names=tuple(out_names), lowering_input_output_aliases=(),
            sim_require_finite=True, sim_require_nnan=True, nc=nc))

    devices = jax.devices()[:n_cores]
    mesh = Mesh(np.asarray(devices), ("core",))
    in_specs = (PartitionSpec("core"),) * (n_params + n_outs)
    out_specs = (PartitionSpec("core"),) * len(out_names)
    sharded = jax.jit(shard_map(_body, mesh=mesh, in_specs=in_specs,
                                out_specs=out_specs, check_rep=False),
                      donate_argnums=donate, keep_unused=True)
    sh = NamedSharding(mesh, PartitionSpec("core"))
    concat_in = [
        jax.device_put(np.concatenate(
            [np.asarray(in_maps[c][in_names[i]]) for c in range(n_cores)], axis=0), sh)
        for i in range(n_params)
    ]
    times = []
    zput = []
    for it in range(4):
        tz = time.perf_counter()
        cz = [jax.device_put(np.zeros((n_cores * z.shape[0], *z.shape[1:]), z.dtype), sh)
              for z in zero_outs]
        jax.block_until_ready(cz)
        t0 = time.perf_counter()
        outs = sharded(*concat_in, *cz)
        jax.block_until_ready(outs)
        t1 = time.perf_counter()
        zput.append(t0 - tz)
        times.append(t1 - t0)
    print("KPERF zeros-put s:", ["%.4f" % t for t in zput])
    print("KPERF exec s:", ["%.4f" % t for t in times])
    globals()["LAST_EXEC_NS"] = int(min(times[1:]) * 1e9)


# revision 19
# speedup vs baseline: 49.5131x; 1.0070x over previous
"""DualGate GNN forward on 8 Trainium2 NeuronCores (Bass/Tile).

Strategy:
  - Nodes sharded across 8 cores (N/8 rows each), weights replicated.
  - Per layer, each core builds a fp16 "table" row-block [TROWS, 128] =
    (h | s=||h||^2 | pad) plus a colsum/N row, AllGathered to a replicated
    table [8*TROWS, 128] in DRAM.
  - Pass A (edges partitioned by DEST core, windows of 128 dest nodes):
    dma_gather of h_src rows + a one-hot-weighted selection matrix M
    (M[e,d] = norm_e * (dest_rel_e == d)) -> PE matmul segmented-sum into
    PSUM window tiles -> agg.  The GCN conv_w multiply is deferred to a
    dense per-node matmul (aggregation is linear).
  - Pass B (edges partitioned by SRC core): same machinery computes
    t_j = sum h_i and u_j = sum s_i over out-edges; then
    ds_j = outdeg_j*s_j - 2 h_j.t_j + u_j  (= sum_out ||h_j - h_i||^2).
  - Gates gs/gq and the node update are dense per-node-tile ops.

Self-contained: derives all sizes from input shapes; no file reads.
"""

import math
import numpy as np

NCORES = 8
P = 128
WG_A = 8    # windows per PSUM group, pass A (8*64 = 512 fp32 = 1 bank)
WG_B = 7    # pass B (7*65 = 455 <= 512)
CALL_MAX_CH = 48  # max chunks (of 128 edges) per dma_gather call
NLAYERS = 2


def _build_pass(dest, src, wgt, trow_of, NLOC, NT, NCHK, CHUNK_ROWS, WG):
    """Per-core slot arrays + a core-equalized static schedule."""
    core = dest // NLOC
    dloc = dest % NLOC
    win = dloc // P
    drel = (dloc % P).astype(np.float32)
    trow = trow_of(src)
    schk = (trow // CHUNK_ROWS).astype(np.int64)
    rel_idx = (trow % CHUNK_ROWS).astype(np.int16)

    NG = math.ceil(NT / WG)

    seg_rank_tab = np.full((NT, NCHK), -1, dtype=np.int64)
    seg_list = []
    for g in range(NG):
        for k in range(NCHK):
            for w in range(g * WG, min((g + 1) * WG, NT)):
                seg_rank_tab[w, k] = len(seg_list)
                seg_list.append((w, k))
    nseg = len(seg_list)

    seg_of_edge = seg_rank_tab[win, schk]
    cnt = np.zeros((NCORES, nseg), dtype=np.int64)
    np.add.at(cnt, (core, seg_of_edge), 1)
    eq_ch = np.ceil(cnt / P).astype(np.int64).max(axis=0)
    # every window needs >= 1 chunk so its PSUM slice gets written
    for w in range(NT):
        if sum(eq_ch[seg_rank_tab[w, k]] for k in range(NCHK)) == 0:
            eq_ch[seg_rank_tab[w, 0]] = 1

    seg_c0 = np.zeros(nseg + 1, dtype=np.int64)
    np.cumsum(eq_ch, out=seg_c0[1:])
    C = int(seg_c0[-1])
    S = C * P

    chunk_win = np.zeros(C, dtype=np.int64)
    for si, (w, k) in enumerate(seg_list):
        chunk_win[seg_c0[si]:seg_c0[si + 1]] = w
    # start/stop at GROUP granularity: start=True zeroes the whole 2KB PSUM
    # bank, so only the first matmul of a group may set it.
    chunk_start = np.zeros(C, dtype=bool)
    chunk_stop = np.zeros(C, dtype=bool)
    for g in range(NG):
        w_lo = g * WG
        w_hi = min((g + 1) * WG, NT) - 1
        gc0 = int(seg_c0[seg_rank_tab[w_lo, 0]])
        gc1 = int(seg_c0[seg_rank_tab[w_hi, NCHK - 1] + 1])
        chunk_start[gc0] = True
        chunk_stop[gc1 - 1] = True

    calls = []
    for g in range(NG):
        ws = list(range(g * WG, min((g + 1) * WG, NT)))
        for k in range(NCHK):
            c0 = int(seg_c0[seg_rank_tab[ws[0], k]])
            c1 = int(seg_c0[seg_rank_tab[ws[-1], k] + 1])
            c = c0
            while c < c1:
                nch = min(CALL_MAX_CH, c1 - c)
                calls.append((c, nch, k, g))
                c += nch

    per_core = []
    for cidx in range(NCORES):
        m = core == cidx
        er = np.nonzero(m)[0]
        order = np.argsort(seg_of_edge[er], kind="stable")
        er = er[order]
        segs = seg_of_edge[er]
        first = np.ones(len(er), dtype=bool)
        if len(er) > 1:
            first[1:] = segs[1:] != segs[:-1]
        startpos = np.nonzero(first)[0]
        runlen = np.diff(np.append(startpos, len(er)))
        within = np.arange(len(er)) - np.repeat(startpos, runlen)
        slot = seg_c0[segs] * P + within

        idx_slots = np.zeros(S, dtype=np.int16)
        dr_slots = np.zeros(S, dtype=np.float32)
        w_slots = np.zeros(S, dtype=np.float32)
        idx_slots[slot] = rel_idx[er]
        dr_slots[slot] = drel[er]
        w_slots[slot] = wgt[er]

        base = idx_slots.reshape(S // 16, 16).T      # [16, S/16]
        idxw = np.ascontiguousarray(np.tile(base, (8, 1)))
        drw = dr_slots.reshape(C, P).T               # [128, C]
        ww = w_slots.reshape(C, P).T
        # host-built selection matrices: M[p, c*128+d] = (drw[p,c]==d)*ww[p,c]
        m_all = ((drw[:, :, None] == np.arange(P, dtype=np.float32))
                 .astype(np.float16) * ww[:, :, None].astype(np.float16))
        m_all = np.ascontiguousarray(m_all.reshape(P, C * P))
        per_core.append(dict(idxw=idxw, m=m_all))

    sched = dict(C=C, calls=calls, chunk_win=chunk_win,
                 chunk_start=chunk_start, chunk_stop=chunk_stop)
    return sched, per_core


def kernel(x, edge_index, enc_w, enc_b, skip_w, conv_w, conv_b, dec_w, dec_b):
    import jax  # noqa: F401  (axon boot side effect)
    import concourse.bass as bass  # noqa: F401
    import concourse.mybir as mybir
    import concourse.tile as tile
    from concourse import bacc
    from concourse.bass_utils import run_bass_kernel_spmd

    FP16, FP32, I16 = mybir.dt.float16, mybir.dt.float32, mybir.dt.int16
    Alu = mybir.AluOpType
    Act = mybir.ActivationFunctionType

    x = np.asarray(x, dtype=np.float32)
    edge_index = np.asarray(edge_index)
    enc_w = np.asarray(enc_w, dtype=np.float32)
    enc_b = np.asarray(enc_b, dtype=np.float32)
    skip_w = np.asarray(skip_w, dtype=np.float32)
    conv_w = np.asarray(conv_w, dtype=np.float32)
    conv_b = np.asarray(conv_b, dtype=np.float32)
    dec_w = np.asarray(dec_w, dtype=np.float32)
    dec_b = np.asarray(dec_b, dtype=np.float32)

    N, F = x.shape
    H = enc_w.shape[1]
    OUTF = dec_w.shape[1]
    assert N % NCORES == 0 and F == 128 and H == 64
    NLOC = N // NCORES
    NT = math.ceil(NLOC / P)
    NLOC_pad = NT * P
    TROWS = NLOC_pad + P
    NCHK = 4
    CHUNK_ROWS = TROWS * NCORES // NCHK
    assert CHUNK_ROWS <= 32767 and TROWS * NCORES % NCHK == 0
    NG_A = math.ceil(NT / WG_A)  # noqa: F841
    TOT_ROWS = TROWS * NCORES

    row = np.asarray(edge_index[0], dtype=np.int64)
    col = np.asarray(edge_index[1], dtype=np.int64)

    deg = np.bincount(col, minlength=N).astype(np.float64) + 1.0
    dinv = 1.0 / np.sqrt(deg)
    norm = (dinv[row] * dinv[col]).astype(np.float32)
    outdeg_np = np.bincount(row, minlength=N).astype(np.float64)
    rdeg_np = (1.0 / (outdeg_np + 1e-10)).astype(np.float32)
    dinv2_np = (dinv * dinv).astype(np.float32)

    def trow_of(n):
        return (n // NLOC) * TROWS + (n % NLOC)

    schedA, pcA = _build_pass(col, row, norm, trow_of, NLOC, NT, NCHK,
                              CHUNK_ROWS, WG_A)
    schedB, pcB = _build_pass(row, col, np.ones(len(row), np.float32), trow_of,
                              NLOC, NT, NCHK, CHUNK_ROWS, WG_B)

    def pack_nodal(v):
        out = np.zeros((NCORES, P, NT), dtype=np.float32)
        for c in range(NCORES):
            padded = np.zeros(NLOC_pad, dtype=np.float32)
            padded[:NLOC] = v[c * NLOC:(c + 1) * NLOC]
            out[c] = padded.reshape(NT, P).T
        return out

    dinv2_pk = pack_nodal(dinv2_np)
    outdeg_pk = pack_nodal(outdeg_np.astype(np.float32))
    rdeg_pk = pack_nodal(rdeg_np)

    x_pk = np.zeros((NCORES, NLOC_pad, F), dtype=np.float32)
    for c in range(NCORES):
        x_pk[c, :NLOC] = x[c * NLOC:(c + 1) * NLOC]

    R_np = np.tile(np.arange(P, dtype=np.float16)[None, :], (P, 1))
    id128_np = np.eye(P, dtype=np.float32)
    ones128_np = np.ones((P, 1), dtype=np.float32)
    ones8_np = np.ones((NCORES, 1), dtype=np.float16)
    ones1x128_np = np.ones((1, P), dtype=np.float16)

    # ---------------- device program ----------------
    nc = bacc.Bacc("TRN2", target_bir_lowering=False, debug=False,
                   num_devices=NCORES)

    def din(name, shape, dt):
        return nc.dram_tensor(name, list(shape), dt, kind="ExternalInput").ap()

    x_e = din("x", (NLOC_pad, F), FP32)
    idxA_e = din("idxA", pcA[0]["idxw"].shape, I16)
    mA_e = din("mA", pcA[0]["m"].shape, FP16)
    idxB_e = din("idxB", pcB[0]["idxw"].shape, I16)
    mB_e = din("mB", pcB[0]["m"].shape, FP16)
    dinv2_e = din("dinv2", (P, NT), FP32)
    outdeg_e = din("outdeg", (P, NT), FP32)
    rdeg_e = din("rdeg", (P, NT), FP32)
    R_e = din("Riota", (P, P), FP16)
    id_e = din("id128", (P, P), FP32)
    ones128_e = din("ones128", (P, 1), FP32)
    ones8_e = din("ones8", (NCORES, 1), FP16)
    ones1x128_e = din("ones1x128", (1, P), FP16)
    encw_e = din("enc_w", (F, H), FP32)
    encb_e = din("enc_b", (H, 1), FP32)
    skipw_e = din("skip_w", (F, H), FP32)
    convw_e = din("conv_w", (H, H), FP32)
    convb_e = din("conv_b", (H, 1), FP32)
    decw_e = din("dec_w", (H, OUTF), FP32)
    decb_e = din("dec_b", (OUTF, 1), FP32)
    out_e = nc.dram_tensor("out", [NLOC_pad, OUTF], FP32,
                           kind="ExternalOutput").ap()
    import os as _os
    KDBG = bool(_os.environ.get("KDBG"))
    if KDBG:
        dbg_h0 = nc.dram_tensor("dbg_h0", [P, NT * H], FP32, kind="ExternalOutput").ap()
        dbg_agg = nc.dram_tensor("dbg_agg", [P, NT * H], FP32, kind="ExternalOutput").ap()
        dbg_tu = nc.dram_tensor("dbg_tu", [P, NT * (H + 1)], FP32, kind="ExternalOutput").ap()
        dbg_gmb = nc.dram_tensor("dbg_gmb", [P, H], FP32, kind="ExternalOutput").ap()
        dbg_h1 = nc.dram_tensor("dbg_h1", [P, NT * H], FP32, kind="ExternalOutput").ap()

    aginp = [nc.dram_tensor(f"aginp{l}", [TROWS, P], FP16)
             for l in range(NLAYERS)]
    tableAG = [nc.dram_tensor(f"tableAG{l}", [TOT_ROWS, P], FP16,
                              addr_space="Shared") for l in range(NLAYERS)]

    with tile.TileContext(nc) as tc:
        with (
            tc.tile_pool(name="const", bufs=1) as cpool,
            tc.tile_pool(name="slab", bufs=1) as slab,
            tc.tile_pool(name="io", bufs=3) as io,
            tc.tile_pool(name="mpool", bufs=4) as mpool,
            tc.tile_pool(name="gpool", bufs=2) as gpool,
            tc.tile_pool(name="meta", bufs=2) as meta,
            tc.tile_pool(name="scr", bufs=3) as scr,
            tc.tile_pool(name="ps_big", bufs=2, space="PSUM") as ps_big,
            tc.tile_pool(name="ps_t", bufs=2, space="PSUM") as ps_t,
            tc.tile_pool(name="ps_f", bufs=2, space="PSUM") as ps_f,
            tc.tile_pool(name="ps_s", bufs=1, space="PSUM") as ps_s,
        ):
            def load_const(ext, shape, dt):
                t = cpool.tile(list(shape), dt, name=ext.tensor.name + "_sb")
                nc.sync.dma_start(out=t[:], in_=ext[:, :])
                return t

            R_t = load_const(R_e, (P, P), FP16)
            id_t = load_const(id_e, (P, P), FP32)
            ones128_t = load_const(ones128_e, (P, 1), FP32)
            ones8_t = load_const(ones8_e, (NCORES, 1), FP16)
            ones1x128_t = load_const(ones1x128_e, (1, P), FP16)
            encw_t = load_const(encw_e, (F, H), FP32)
            encb_t = load_const(encb_e, (H, 1), FP32)
            skipw_t = load_const(skipw_e, (F, H), FP32)
            convw_t = load_const(convw_e, (H, H), FP32)
            convb_t = load_const(convb_e, (H, 1), FP32)
            decw_t = load_const(decw_e, (H, OUTF), FP32)
            decb_t = load_const(decb_e, (OUTF, 1), FP32)
            dinv2_t = load_const(dinv2_e, (P, NT), FP32)
            outdeg_t = load_const(outdeg_e, (P, NT), FP32)
            rdeg_t = load_const(rdeg_e, (P, NT), FP32)

            h_sl = slab.tile([P, NT * H], FP32)
            xskip_sl = slab.tile([P, NT * H], FP32)
            agg_sl = slab.tile([P, NT * H], FP32)
            tu_sl = slab.tile([P, NT * (H + 1)], FP32)
            s_sl = slab.tile([P, NT], FP32)
            gmb_sl = slab.tile([P, H], FP32)
            zero16 = cpool.tile([P, P], FP16)
            nc.vector.memset(zero16[:], 0)

            def hsl(t):
                return h_sl[:, t * H:(t + 1) * H]

            def xssl(t):
                return xskip_sl[:, t * H:(t + 1) * H]

            def aggsl(t):
                return agg_sl[:, t * H:(t + 1) * H]

            def tsl(t):
                return tu_sl[:, t * (H + 1):t * (H + 1) + H]

            def usl(t):
                return tu_sl[:, t * (H + 1) + H:(t + 1) * (H + 1)]

            def pt_tile():
                return ps_t.tile([P, P], FP32, space="PSUM", tag="pt", name="pt")

            def pf_tile():
                return ps_f.tile([P, P], FP32, space="PSUM", tag="pf", name="pf")

            # ---------- encoder ----------
            for t in range(NT):
                xt = io.tile([P, F], FP32, tag="xt")
                nc.sync.dma_start(out=xt[:], in_=x_e[t * P:(t + 1) * P, :])
                pxT = pt_tile()
                nc.tensor.transpose(pxT[:], xt[:], id_t[:])
                xT = scr.tile([P, P], FP32, tag="xT")
                nc.vector.tensor_copy(xT[:], pxT[:])

                pf = pf_tile()
                nc.tensor.matmul(pf[0:H, :], lhsT=encw_t[:], rhs=xT[:],
                                 start=True, stop=True)
                hT = scr.tile([H, P], FP32, tag="hT")
                nc.scalar.activation(hT[:], pf[0:H, :], Act.Relu,
                                     bias=encb_t[:, 0:1], scale=1.0)
                pb = pt_tile()
                nc.tensor.transpose(pb[:, 0:H], hT[:], id_t[0:H, 0:H])
                nc.vector.tensor_copy(hsl(t), pb[:, 0:H])

                pf2 = pf_tile()
                nc.tensor.matmul(pf2[0:H, :], lhsT=skipw_t[:], rhs=xT[:],
                                 start=True, stop=True)
                sT = scr.tile([H, P], FP32, tag="hT")
                nc.vector.tensor_copy(sT[:], pf2[0:H, :])
                pb2 = pt_tile()
                nc.tensor.transpose(pb2[:, 0:H], sT[:], id_t[0:H, 0:H])
                nc.vector.tensor_copy(xssl(t), pb2[:, 0:H])

            if KDBG:
                nc.sync.dma_start(out=dbg_h0[:, :], in_=h_sl[:])

            # ---------- layers ----------
            _rep = int(_os.environ.get("KREPEAT", "1"))
            for _li in range(NLAYERS * _rep):
                l = _li % NLAYERS
                # table build
                pcs = ps_s.tile([1, H], FP32, space="PSUM", tag="pcs")
                for t in range(NT):
                    sq = scr.tile([P, H], FP32, tag="sq")
                    nc.vector.tensor_mul(sq[:], hsl(t), hsl(t))
                    nc.vector.tensor_reduce(s_sl[:, t:t + 1], sq[:],
                                            axis=mybir.AxisListType.X,
                                            op=Alu.add)
                    tb = io.tile([P, P], FP16, tag="tb")
                    nc.vector.tensor_copy(tb[:, 0:H], hsl(t))
                    nc.vector.tensor_copy(tb[:, H:H + 1], s_sl[:, t:t + 1])
                    nc.vector.tensor_copy(tb[:, H + 1:P], zero16[:, H + 1:P])
                    nc.sync.dma_start(out=aginp[l][t * P:(t + 1) * P, :],
                                      in_=tb[:])
                    nc.tensor.matmul(pcs[:], lhsT=ones128_t[:], rhs=hsl(t),
                                     start=(t == 0), stop=(t == NT - 1),
                                     skip_group_check=True)
                csum = scr.tile([1, H], FP32, tag="csum")
                nc.scalar.activation(csum[:], pcs[:], Act.Copy, bias=0.0,
                                     scale=1.0 / N)
                csrow = io.tile([1, P], FP16, tag="csrow")
                nc.vector.tensor_copy(csrow[:, 0:H], csum[:])
                nc.vector.tensor_copy(csrow[:, H:P], zero16[0:1, H:P])
                nc.sync.dma_start(out=aginp[l][NLOC_pad:NLOC_pad + 1, :],
                                  in_=csrow[:])
                nc.sync.dma_start(out=aginp[l][NLOC_pad + 1:NLOC_pad + P, :],
                                  in_=zero16[0:P - 1, :])

                nc.gpsimd.collective_compute(
                    "AllGather", Alu.bypass,
                    replica_groups=[list(range(NCORES))],
                    ins=[aginp[l][:, :]],
                    outs=[tableAG[l][:, :]],
                )

                # gmean from the 8 colsum/N rows
                gm8 = scr.tile([NCORES, H], FP16, tag="gm8")
                tview = tableAG[l][:, :].rearrange("(r t) e -> r t e", r=NCORES)
                nc.sync.dma_start(out=gm8[:], in_=tview[:, NLOC_pad, 0:H])
                pgm = ps_s.tile([1, H], FP32, space="PSUM", tag="pcs")
                nc.tensor.matmul(pgm[:], lhsT=ones8_t[:], rhs=gm8[:],
                                 start=True, stop=True)
                gmrow = scr.tile([1, H], FP16, tag="gmrow")
                nc.vector.tensor_copy(gmrow[:], pgm[:])
                pgb = pf_tile()
                nc.tensor.matmul(pgb[:, 0:H], lhsT=ones1x128_t[:], rhs=gmrow[:],
                                 start=True, stop=True)
                nc.vector.tensor_copy(gmb_sl[:], pgb[:, 0:H])

                # ---- gather/segsum passes ----
                def run_pass(sched, idx_e, m_e, WG, FW, out_slab_fn):
                    chunk_win = sched["chunk_win"]
                    cst, csp = sched["chunk_start"], sched["chunk_stop"]
                    state = dict(g=-1, pgrp=None, gw=0)

                    def evac():
                        if _os.environ.get("KGONLY"):
                            return
                        g, gw, pg = state["g"], state["gw"], state["pgrp"]
                        w0 = g * WG
                        nc.scalar.activation(out_slab_fn(w0, gw),
                                             pg[:, 0:gw * FW], Act.Copy,
                                             bias=0.0, scale=1.0)

                    for (c0, nch, k, g) in sched["calls"]:
                        if g != state["g"]:
                            if state["pgrp"] is not None:
                                evac()
                            state["g"] = g
                            state["gw"] = min(WG, NT - g * WG)
                            state["pgrp"] = ps_big.tile([P, 512], FP32, space="PSUM", tag="pgrp", name="pgrp")
                        pgrp = state["pgrp"]
                        L = nch * P
                        idx_t = meta.tile([P, CALL_MAX_CH * 8], I16, tag="idx")
                        nc.sync.dma_start(out=idx_t[:, 0:nch * 8],
                                          in_=idx_e[:, c0 * 8:(c0 + nch) * 8])
                        m_all_t = meta.tile([P, CALL_MAX_CH * P], FP16,
                                            tag="mld")
                        nc.sync.dma_start(out=m_all_t[:, 0:nch * P],
                                          in_=m_e[:, c0 * P:(c0 + nch) * P])
                        g_t = gpool.tile([P, CALL_MAX_CH * P], FP16, tag="gt")
                        g3 = g_t[:].rearrange("p (c e) -> p c e", c=CALL_MAX_CH)
                        nc.gpsimd.dma_gather(
                            out_ap=g3[:, 0:nch, :],
                            in_ap=tableAG[l][k * CHUNK_ROWS:(k + 1) * CHUNK_ROWS, :],
                            idxs_ap=idx_t[:, 0:nch * 8],
                            num_idxs=L, num_idxs_reg=L, elem_size=P,
                            single_packet=False,
                        )
                        if _os.environ.get("KGONLY"):
                            continue
                        for j in range(nch):
                            cj = c0 + j
                            w = int(chunk_win[cj])
                            wloc = w - g * WG
                            nc.tensor.matmul(
                                pgrp[:, wloc * FW:wloc * FW + FW],
                                lhsT=m_all_t[:, j * P:(j + 1) * P],
                                rhs=g3[:, j, 0:FW],
                                start=bool(cst[cj]), stop=bool(csp[cj]),
                                skip_group_check=True)
                    if state["pgrp"] is not None:
                        evac()

                def aggout(w0, gw):
                    return agg_sl[:, w0 * H:(w0 + gw) * H]

                def tuout(w0, gw):
                    return tu_sl[:, w0 * (H + 1):(w0 + gw) * (H + 1)]

                if _os.environ.get("KNOPASS") or _os.environ.get("KGONLY"):
                    nc.vector.memset(agg_sl[:], 0)
                    nc.vector.memset(tu_sl[:], 0)
                if not _os.environ.get("KNOPASS"):
                    run_pass(schedA, idxA_e, mA_e, WG_A, H, aggout)
                    run_pass(schedB, idxB_e, mB_e, WG_B, H + 1, tuout)
                if KDBG and l == 0:
                    nc.sync.dma_start(out=dbg_agg[:, :], in_=agg_sl[:])
                    nc.sync.dma_start(out=dbg_tu[:, :], in_=tu_sl[:])
                    nc.sync.dma_start(out=dbg_gmb[:, :], in_=gmb_sl[:])

                # ---- dense per-tile update ----
                for t in range(NT if not _os.environ.get("KNODENSE") else 0):
                    q = scr.tile([P, H], FP32, tag="q")
                    nc.vector.tensor_scalar(out=q[:], in0=hsl(t),
                                            scalar1=dinv2_t[:, t:t + 1],
                                            scalar2=None, op0=Alu.mult)
                    nc.vector.tensor_add(q[:], q[:], aggsl(t))
                    pq = pt_tile()
                    nc.tensor.transpose(pq[0:H, :], q[:], id_t[:])
                    qT = scr.tile([H, P], FP32, tag="qT")
                    nc.vector.tensor_copy(qT[:], pq[0:H, :])
                    pxa = pf_tile()
                    nc.tensor.matmul(pxa[0:H, :], lhsT=convw_t[:], rhs=qT[:],
                                     start=True, stop=True)
                    xaT = scr.tile([H, P], FP32, tag="xaT")
                    nc.scalar.activation(xaT[:], pxa[0:H, :], Act.Relu,
                                         bias=convb_t[:, 0:1], scale=1.0)
                    pxa2 = pt_tile()
                    nc.tensor.transpose(pxa2[:, 0:H], xaT[:], id_t[0:H, 0:H])
                    xagg = scr.tile([P, H], FP32, tag="xagg")
                    nc.vector.tensor_copy(xagg[:], pxa2[:, 0:H])

                    ht = scr.tile([P, H], FP32, tag="ht")
                    nc.vector.tensor_mul(ht[:], hsl(t), tsl(t))
                    hdot = scr.tile([P, 1], FP32, tag="hdot")
                    nc.vector.tensor_reduce(hdot[:], ht[:],
                                            axis=mybir.AxisListType.X,
                                            op=Alu.add)
                    ds = scr.tile([P, 1], FP32, tag="ds")
                    nc.vector.tensor_scalar(out=ds[:], in0=hdot[:],
                                            scalar1=-2.0, scalar2=None,
                                            op0=Alu.mult)
                    nc.vector.tensor_add(ds[:], ds[:], usl(t))
                    os_ = scr.tile([P, 1], FP32, tag="os")
                    nc.vector.tensor_scalar(out=os_[:], in0=s_sl[:, t:t + 1],
                                            scalar1=outdeg_t[:, t:t + 1],
                                            scalar2=None, op0=Alu.mult)
                    nc.vector.tensor_add(ds[:], ds[:], os_[:])
                    gs = scr.tile([P, 1], FP32, tag="gs")
                    nc.scalar.activation(gs[:], ds[:], Act.Tanh,
                                         scale=rdeg_t[:, t:t + 1])

                    df = scr.tile([P, H], FP32, tag="df")
                    nc.vector.tensor_sub(df[:], hsl(t), gmb_sl[:])
                    av = scr.tile([P, H], FP32, tag="av")
                    nc.scalar.activation(av[:], df[:], Act.Abs)
                    cv = scr.tile([P, H], FP32, tag="cv")
                    nc.scalar.activation(cv[:], av[:], Act.Sqrt)
                    nc.vector.tensor_mul(cv[:], cv[:], av[:])   # a^1.5
                    nc.vector.tensor_mul(cv[:], cv[:], av[:])   # a^2.5
                    dq = scr.tile([P, 1], FP32, tag="dq")
                    nc.vector.tensor_reduce(dq[:], cv[:],
                                            axis=mybir.AxisListType.X,
                                            op=Alu.add)
                    gq = scr.tile([P, 1], FP32, tag="gq")
                    nc.scalar.activation(gq[:], dq[:], Act.Tanh)
                    nc.vector.tensor_scalar(out=gq[:], in0=gq[:], scalar1=-1.0,
                                            scalar2=1.0, op0=Alu.mult,
                                            op1=Alu.add)

                    den = scr.tile([P, 1], FP32, tag="den")
                    nc.vector.tensor_add(den[:], gs[:], gq[:])
                    nc.vector.tensor_scalar(out=den[:], in0=den[:],
                                            scalar1=1.0, scalar2=None,
                                            op0=Alu.add)
                    rec = scr.tile([P, 1], FP32, tag="rec")
                    nc.vector.reciprocal(rec[:], den[:])

                    u1 = scr.tile([P, H], FP32, tag="u1")
                    nc.vector.tensor_scalar(out=u1[:], in0=xagg[:],
                                            scalar1=gs[:, 0:1], scalar2=None,
                                            op0=Alu.mult)
                    u2 = scr.tile([P, H], FP32, tag="u2")
                    nc.vector.tensor_scalar(out=u2[:], in0=xssl(t),
                                            scalar1=gq[:, 0:1], scalar2=None,
                                            op0=Alu.mult)
                    nc.vector.tensor_add(u1[:], u1[:], u2[:])
                    nc.vector.tensor_add(u1[:], u1[:], hsl(t))
                    nc.vector.tensor_scalar(out=hsl(t), in0=u1[:],
                                            scalar1=rec[:, 0:1], scalar2=None,
                                            op0=Alu.mult)

                if KDBG and l == 0:
                    nc.sync.dma_start(out=dbg_h1[:, :], in_=h_sl[:])

            # ---------- decoder ----------
            for t in range(NT):
                ph = pt_tile()
                nc.tensor.transpose(ph[0:H, :], hsl(t), id_t[:])
                hT2 = scr.tile([H, P], FP32, tag="qT")
                nc.vector.tensor_copy(hT2[:], ph[0:H, :])
                po = pf_tile()
                nc.tensor.matmul(po[0:OUTF, :], lhsT=decw_t[:], rhs=hT2[:],
                                 start=True, stop=True)
                oT = scr.tile([OUTF, P], FP32, tag="oT")
                nc.vector.tensor_scalar(out=oT[:], in0=po[0:OUTF, :],
                                        scalar1=decb_t[:, 0:1], scalar2=None,
                                        op0=Alu.add)
                pob = pt_tile()
                nc.tensor.transpose(pob[:, 0:OUTF], oT[:], id_t[0:OUTF, 0:OUTF])
                ot = io.tile([P, OUTF], FP32, tag="ot")
                nc.vector.tensor_copy(ot[:], pob[:, 0:OUTF])
                nc.sync.dma_start(out=out_e[t * P:(t + 1) * P, :], in_=ot[:])

    nc.compile()

    in_maps = []
    for c in range(NCORES):
        in_maps.append({
            "x": x_pk[c],
            "idxA": pcA[c]["idxw"], "mA": pcA[c]["m"],
            "idxB": pcB[c]["idxw"], "mB": pcB[c]["m"],
            "dinv2": dinv2_pk[c], "outdeg": outdeg_pk[c], "rdeg": rdeg_pk[c],
            "Riota": R_np, "id128": id128_np, "ones128": ones128_np,
            "ones8": ones8_np, "ones1x128": ones1x128_np,
            "enc_w": enc_w, "enc_b": enc_b.reshape(H, 1),
            "skip_w": skip_w, "conv_w": conv_w,
            "conv_b": conv_b.reshape(H, 1),
            "dec_w": dec_w, "dec_b": dec_b.reshape(OUTF, 1),
        })

    import os
    if os.environ.get("KSIM"):
        from concourse import bass_interp
        sim = bass_interp.MultiCoreSim(nc, num_cores=NCORES)
        for c in range(NCORES):
            for k, v in in_maps[c].items():
                sim.cores[c].tensor(k)[:] = v
        sim.simulate(check_with_hw=False)
        if KDBG:
            kernel.dbg = [{k: np.array(sim.cores[c].tensor(k)) for k in
                           ("dbg_h0", "dbg_agg", "dbg_tu", "dbg_gmb", "dbg_h1")}
                          for c in range(NCORES)]
        out = np.concatenate([np.array(sim.cores[c].tensor("out"))[:NLOC]
                              for c in range(NCORES)], axis=0)
        return out.astype(np.float32)

    import time as _time
    if os.environ.get("KPERF"):
        _perf_time(nc, in_maps)
    t0 = _time.perf_counter()
    res = run_bass_kernel_spmd(nc, in_maps, list(range(NCORES)))
    t1 = _time.perf_counter()
    globals()["LAST_RUN_S"] = t1 - t0
    if os.environ.get("KTIME"):
        t2 = _time.perf_counter()
        res = run_bass_kernel_spmd(nc, in_maps, list(range(NCORES)))
        t3 = _time.perf_counter()
        globals()["LAST_RUN2_S"] = t3 - t2
        globals()["LAST_EXEC_NS"] = int((t3 - t2) * 1e9)
    out = np.concatenate([res.results[c]["out"][:NLOC]
                          for c in range(NCORES)], axis=0)
    return out.astype(np.float32)


def _perf_time(nc, in_maps):
    """Time warm on-device executions with inputs pre-staged on devices."""
    import time
    import numpy as np
    import jax
    from jax.sharding import Mesh, PartitionSpec, NamedSharding
    from jax.experimental.shard_map import shard_map
    import concourse.mybir as mybir
    from concourse import bass2jax
    from concourse.bass2jax import _bass_exec_p, partition_id_tensor

    bass2jax.install_neuronx_cc_hook()
    n_cores = len(in_maps)
    partition_name = nc.partition_id_tensor.name if nc.partition_id_tensor else None
    in_names, out_names, out_avals, zero_outs = [], [], [], []
    for alloc in nc.m.functions[0].allocations:
        if not isinstance(alloc, mybir.MemoryLocationSet):
            continue
        name = alloc.memorylocations[0].name
        if alloc.kind == "ExternalInput":
            if name != partition_name:
                in_names.append(name)
        elif alloc.kind == "ExternalOutput":
            shape = tuple(alloc.tensor_shape)
            dtype = mybir.dt.np(alloc.dtype)
            out_names.append(name)
            out_avals.append(jax.core.ShapedArray(shape, dtype))
            zero_outs.append(np.zeros(shape, dtype))
    n_params = len(in_names)
    n_outs = len(out_avals)
    in_names.extend(out_names)
    if partition_name is not None:
        in_names.append(partition_name)
    donate = tuple(range(n_params, n_params + n_outs))

    def _body(*args):
        operands = list(args)
        if partition_name is not None:
            operands.append(partition_id_tensor())
        return tuple(_bass_exec_p.bind(
            *operands, out_avals=tuple(out_avals), in_names=tuple(in_names),
            out_names=tuple(out_names), lowering_input_output_aliases=(),
            sim_require_finite=True, sim_require_nnan=True, nc=nc))

    devices = jax.devices()[:n_cores]
    mesh = Mesh(np.asarray(devices), ("core",))
    in_specs = (PartitionSpec("core"),) * (n_params + n_outs)
    out_specs = (PartitionSpec("core"),) * len(out_names)
    sharded = jax.jit(shard_map(_body, mesh=mesh, in_specs=in_specs,
                                out_specs=out_specs, check_rep=False),
                      donate_argnums=donate, keep_unused=True)
    sh = NamedSharding(mesh, PartitionSpec("core"))
    concat_in = [
        jax.device_put(np.concatenate(
            [np.asarray(in_maps[c][in_names[i]]) for c in range(n_cores)], axis=0), sh)
        for i in range(n_params)
    ]
    times = []
    zput = []
    for it in range(4):
        tz = time.perf_counter()
        cz = [jax.device_put(np.zeros((n_cores * z.shape[0], *z.shape[1:]), z.dtype), sh)
              for z in zero_outs]
        jax.block_until_ready(cz)
        t0 = time.perf_counter()
        outs = sharded(*concat_in, *cz)
        jax.block_until_ready(outs)
        t1 = time.perf_counter()
        zput.append(t0 - tz)
        times.append(t1 - t0)
    print("KPERF zeros-put s:", ["%.4f" % t for t in zput])
    print("KPERF exec s:", ["%.4f" % t for t in times])
    globals()["LAST_EXEC_NS"] = int(min(times[1:]) * 1e9)
